# revision 1
# baseline (speedup 1.0000x reference)
"""EquivariantGraphConvolution (EGNN layer) on 8 Trainium2 NeuronCores.

Strategy (v2)
-------------
Nodes are range-partitioned across the 8 cores (6250 nodes each); every edge is
owned by the core that owns its *start* node, so the per-start segment sums are
core-local and no collective is needed.  Per core, edges are bucketed by
(128-node start block, end<25088) cells and padded to a static layout of
49 blocks x (12+12) chunks x 128 edges.

End-side first-layer partials are fetched with ONE transposed dma_gather per
block from a packed bf16 table T2p whose row r holds [P2[r] | P2[r+25088]]
(so both 64-value halves of the 256-byte gather element are useful and the
start/end split needs no second gather).  Start-side partials P1 are injected
with one-hot matmuls (no gather): the one-hot [node x edge] matrix is built
on-chip from a K=1 broadcast matmul plus a DVE is_equal.  The edge MLP runs
feature-major with 2-group block-diagonal matmuls; the feature->edge-major
flip fuses msg+coord-hidden into a single matmul per 128-edge chunk, and the
segment sum is one matmul per chunk (msg|coord merged stationary).
"""
import sys
sys.path.insert(0, "/opt/trn_rl_repo")
import contextlib
import os
import numpy as np

import concourse.bass as bass
import concourse.bacc as bacc
import concourse.mybir as mybir
import concourse.tile as tile
from concourse.bass_utils import run_bass_kernel_spmd

f32 = mybir.dt.float32
bf16 = mybir.dt.bfloat16
i16 = mybir.dt.int16
i32 = mybir.dt.int32
AF = mybir.ActivationFunctionType
OP = mybir.AluOpType

# ---- problem constants (hardcoded per contract) ----
N = 50000
E = 1_000_000
H = 64
EF = 16
NCORES = 8
NPC = N // NCORES          # 6250 nodes per core
NBLK = 49                  # 128-node blocks per core (49*128 = 6272 >= 6250)
NPAD = NBLK * 128          # 6272
TS2 = 25088                # packed-table split; NFULL = 2*TS2
NFULL = 2 * TS2            # 50176 = 128*392
CPR = 12                   # chunks per (block, range) cell
CELL = CPR * 128           # 1536 edge slots per cell
BLKE = 2 * CELL            # 3072 edge slots per block
ES = NBLK * BLKE           # 150528 edge slots per core
NSUP = 3                   # supertiles per block (512-edge groups per cell)

SINGLE_PACKET = os.environ.get("EGNN_SP", "0") == "1"
# CoreSim does not implement Silu; substitute Tanh for structural sim checks.
SIM_ACT = os.environ.get("EGNN_SIMACT", "0") == "1"
AF_SILU = None  # set below once AF exists

AF_SILU = AF.Tanh if SIM_ACT else AF.Silu

_cache = {}


def _f(x):
    return np.ascontiguousarray(x, np.float32)


def _prep_weights(inp):
    """Small weight/constant tensors, identical on all cores."""
    W_e1 = _f(inp["W_e1"])           # [145, 64]
    w = {}
    w["w1a"] = W_e1[0:64].copy()                     # [64, 64] start partial
    w1b_lo = np.zeros((64, 128), np.float32)
    w1b_lo[:, 0:64] = W_e1[64:128]
    w1b_hi = np.zeros((64, 128), np.float32)
    w1b_hi[:, 64:128] = W_e1[64:128]
    w["w1blo"] = w1b_lo
    w["w1bhi"] = w1b_hi
    wef = np.zeros((34, 128), np.float32)
    wef[0:16, 0:64] = W_e1[129:145]
    wef[16:17, 0:64] = W_e1[128:129]
    wef[17:33, 64:128] = W_e1[129:145]
    wef[33:34, 64:128] = W_e1[128:129]
    w["wefcdn"] = wef
    injA = np.zeros((128, 64), np.float32)
    injA[0:64] = np.eye(64)
    w["injA"] = injA                                  # low edges: P2 in rows 0:64
    injB = np.zeros((128, 64), np.float32)
    injB[64:128] = np.eye(64)
    w["injB"] = injB                                  # high edges: P2 in rows 64:128
    W_e2 = _f(inp["W_e2"]); W_c1 = _f(inp["W_c1"])
    bd = np.zeros((128, 128), np.float32)
    bd[0:64, 0:64] = W_e2; bd[64:128, 64:128] = W_e2
    w["wde2"] = bd
    # swapped block-diagonal: out rows 0:64 = W_c1^T msgB, rows 64:128 = W_c1^T msgA
    bcx = np.zeros((128, 128), np.float32)
    bcx[64:128, 0:64] = W_c1
    bcx[0:64, 64:128] = W_c1
    w["wdc1x"] = bcx
    W_i = _f(inp["W_i"]); W_c2 = _f(inp["W_c2"])
    # mcA = [msgA ; chA]  -> out cols: 0:64 msg, 64 gate logit, 65 coord weight
    ra = np.zeros((128, 66), np.float32)
    ra[0:64, 0:64] = np.eye(64); ra[0:64, 64:65] = W_i; ra[64:128, 65:66] = W_c2
    w["rawcA"] = ra
    # mcB = [chB ; msgB]
    rb = np.zeros((128, 66), np.float32)
    rb[64:128, 0:64] = np.eye(64); rb[64:128, 64:65] = W_i; rb[0:64, 65:66] = W_c2
    w["rawcB"] = rb
    w["wn1"] = _f(inp["W_n1"])                        # [128, 64]
    w["wn2"] = _f(inp["W_n2"])
    w["wv1"] = _f(inp["W_v1"])
    w["wv2"] = _f(inp["W_v2"])
    w["eye64"] = np.eye(64, dtype=np.float32)

    b_e1 = _f(inp["b_e1"]); b_e2 = _f(inp["b_e2"]); b_c1 = _f(inp["b_c1"])
    w["be1s"] = np.concatenate([b_e1, b_e1]).reshape(128, 1)
    w["be2s"] = np.concatenate([b_e2, b_e2]).reshape(128, 1)
    w["bc1s"] = np.concatenate([b_c1, b_c1]).reshape(128, 1)
    w["bih"] = np.full((128, 1), 0.5 * float(np.asarray(inp["b_i"]).ravel()[0]), np.float32)
    w["bn1c"] = _f(inp["b_n1"]).reshape(64, 1)
    w["bn2c"] = _f(inp["b_n2"]).reshape(64, 1)
    w["bv1c"] = _f(inp["b_v1"]).reshape(64, 1)
    w["bv2c"] = np.full((1, 1), float(np.asarray(inp["b_v2"]).ravel()[0]), np.float32)
    return w


def _wrap16(idx_slots):
    """[n] int16 -> [128, n/16]: index i at (i%16, i//16), replicated x8."""
    n = idx_slots.shape[0]
    base = idx_slots.reshape(n // 16, 16).T          # [16, n/16]
    return np.tile(base, (8, 1)).astype(np.int16)    # [128, n/16]


def _prep_core(c, start, end, ef, nfi, cd_all, cdn_all, invcnt_all):
    lo, hi = c * NPC, (c + 1) * NPC
    sel = (start >= lo) & (start < hi)
    eo = np.nonzero(sel)[0]
    s_loc = (start[eo] - lo).astype(np.int64)
    e_g = end[eo].astype(np.int64)
    blk = s_loc >> 7
    rbit = (e_g >= TS2).astype(np.int64)
    cellid = blk * 2 + rbit
    order = np.argsort(cellid, kind="stable")
    eo = eo[order]; s_loc = s_loc[order]; e_g = e_g[order]
    blk = blk[order]; rbit = rbit[order]; cellid = cellid[order]
    counts = np.bincount(cellid, minlength=2 * NBLK)
    if counts.max() > CELL:
        raise RuntimeError(f"cell overflow: {counts.max()} > {CELL}")
    starts_ = np.zeros(2 * NBLK, np.int64)
    starts_[1:] = np.cumsum(counts)[:-1]
    within = np.arange(len(eo)) - starts_[cellid]
    slots = blk * BLKE + rbit * CELL + within

    g2 = np.zeros(ES, np.int64)
    lid = np.full(ES, -1.0, np.float32)
    cds = np.zeros((ES, 3), np.float32)
    cdns = np.zeros(ES, np.float32)
    efs = np.zeros((ES, EF), np.float32)
    g2[slots] = e_g - rbit * TS2
    lid[slots] = (s_loc & 127).astype(np.float32)
    cds[slots] = cd_all[eo]
    cdns[slots] = cdn_all[eo]
    efs[slots] = ef[eo]

    bfdt = mybir.dt.np(bf16)
    d = {}
    a2 = g2.reshape(NBLK, BLKE)
    d["g2i"] = np.stack([_wrap16(a2[b]) for b in range(NBLK)])          # [NBLK,128,192]
    ar = np.arange(128, dtype=np.float32)
    lidb = lid.reshape(NBLK, BLKE)
    # gather-orientation one-hot [node-partition x edge]
    d["ohTg"] = (lidb[:, None, :] == ar[None, :, None]).astype(bfdt)     # [NBLK,128,BLKE]
    # scatter-orientation one-hot [edge-partition x chunk x node]
    lidc = lid.reshape(NBLK, 24, 128)
    d["ohtS"] = (lidc.transpose(0, 2, 1)[:, :, :, None]
                 == ar[None, None, None, :]).astype(bfdt)                # [NBLK,128,24,128]
    d["cdem"] = cds.reshape(NBLK, 24, 128, 3).transpose(0, 2, 1, 3).copy()  # [NBLK,128,24,3]
    efr = efs.reshape(NBLK, 2, NSUP, 512, EF)
    cdnr = cdns.reshape(NBLK, 2, NSUP, 512)
    eA = np.concatenate([efr[:, 0].transpose(0, 1, 3, 2),
                         cdnr[:, 0][:, :, None, :]], axis=2)             # [NBLK,3,17,512]
    eB = np.concatenate([efr[:, 1].transpose(0, 1, 3, 2),
                         cdnr[:, 1][:, :, None, :]], axis=2)
    ecat = np.concatenate([eA, eB], axis=2)                              # [NBLK,3,34,512]
    d["efcdn"] = np.ascontiguousarray(
        ecat.transpose(0, 2, 1, 3).reshape(NBLK, 34, NSUP * 512)).astype(bfdt)

    nm = np.zeros((NPAD, 70), np.float32)
    nm[0:NPC] = nfi[lo:hi]
    d["node_nm"] = nm.reshape(NBLK, 128, 70).transpose(1, 0, 2).reshape(128, NBLK * 70).copy()
    ic = np.ones(NPAD, np.float32)
    ic[0:NPC] = invcnt_all[lo:hi]
    d["invcnt"] = ic.reshape(NBLK, 128).T.copy()                         # [128, NBLK]
    nl = np.zeros((64, NPAD), np.float32)
    nl[:, 0:NPC] = nfi[lo:hi, 6:70].T
    d["nfT_local"] = nl
    return d


def _build_program():
    STAGE = int(os.environ.get("EGNN_STAGE", "5"))
    nc = bacc.Bacc("TRN2", target_bir_lowering=False, debug=False,
                   enable_asserts=False, num_devices=NCORES)

    def din(name, shape, dt=f32):
        return nc.dram_tensor(name, list(shape), dt, kind="ExternalInput").ap()

    nfT_full = din("nfT_full", [64, NFULL], bf16)
    g2i = din("g2i", [NBLK, 128, 192], i16)
    ohTg_d = din("ohTg", [NBLK, 128, BLKE], bf16)
    ohtS_d = din("ohtS", [NBLK, 128, 24, 128], bf16)
    cdem_d = din("cdem", [NBLK, 128, 24, 3])
    efcdn_d = din("efcdn", [NBLK, 34, NSUP * 512], bf16)
    invcnt_d = din("invcnt", [128, NBLK])
    node_nm_d = din("node_nm", [128, NBLK * 70])
    nfT_loc_d = din("nfT_local", [64, NPAD])
    wnames = ["w1a", "w1blo", "w1bhi", "wefcdn", "injA", "injB", "wde2",
              "wdc1x", "rawcA", "rawcB", "wn1", "wn2", "wv1", "wv2",
              "eye64",
              "be1s", "be2s", "bc1s", "bih", "bn1c", "bn2c", "bv1c", "bv2c"]
    wshapes = {"w1a": [64, 64], "w1blo": [64, 128], "w1bhi": [64, 128],
               "wefcdn": [34, 128], "injA": [128, 64], "injB": [128, 64],
               "wde2": [128, 128], "wdc1x": [128, 128],
               "rawcA": [128, 66], "rawcB": [128, 66],
               "wn1": [128, 64], "wn2": [64, 64],
               "wv1": [64, 64], "wv2": [64, 1], "eye64": [64, 64],
               "be1s": [128, 1], "be2s": [128, 1], "bc1s": [128, 1],
               "bih": [128, 1], "bn1c": [64, 1], "bn2c": [64, 1],
               "bv1c": [64, 1], "bv2c": [1, 1]}
    wd = {n: din(n, wshapes[n]) for n in wnames}
    out_d = nc.dram_tensor("out", [NPAD, 70], f32, kind="ExternalOutput").ap()
    T2p = nc.dram_tensor("T2p", [TS2, 128], bf16).ap()

    # weights that are matmul operands in the bf16 pipeline
    BF_W = ("w1blo", "w1bhi", "wefcdn", "injA", "injB", "wde2", "wdc1x",
            "rawcA", "rawcB")

    with tile.TileContext(nc) as tc, contextlib.ExitStack() as ctx:
        wpool = ctx.enter_context(tc.tile_pool(name="w", bufs=1))
        wt = {}
        for n in wnames:
            dt = bf16 if n in BF_W else f32
            t = wpool.tile(wshapes[n], dt, name=f"wt_{n}")
            if dt == f32:
                nc.sync.dma_start(t[:], wd[n][:])
            else:
                tf = wpool.tile(wshapes[n], f32, name=f"wtf_{n}")
                nc.sync.dma_start(tf[:], wd[n][:])
                nc.vector.tensor_copy(t[:], tf[:])
            wt[n] = t
        node_nm = wpool.tile([128, NBLK * 70], f32, name="node_nm")
        nc.sync.dma_start(node_nm[:], node_nm_d[:])
        invcnt = wpool.tile([128, NBLK], f32, name="invcnt")
        nc.sync.dma_start(invcnt[:], invcnt_d[:])
        nfT_loc = wpool.tile([64, NPAD], f32, name="nfT_loc")
        nc.sync.dma_start(nfT_loc[:], nfT_loc_d[:])
        vscale = wpool.tile([128, NBLK], f32, name="vscale")
        aggsb = wpool.tile([128, NPAD], f32, name="aggsb")

        # ---------- Phase A: packed end-partial table T2p ----------
        with tc.tile_pool(name="pan", bufs=1) as pan, \
             tc.tile_pool(name="pa", bufs=2) as pa, \
             tc.tile_pool(name="pap", bufs=4, space="PSUM") as pap:
            nff = pan.tile([64, NFULL], bf16, name="nff", tag="nff")
            nc.sync.dma_start(nff[:], nfT_full[:])
            for g in range(NBLK):          # 49 groups of 4 row-blocks
                stg = pa.tile([128, 4, 128], bf16, name=f"stg{g}", tag="stg")
                for jj in range(4):
                    j = g * 4 + jj
                    ps = pap.tile([128, 128], f32, name=f"aps{j}", tag="aps")
                    nc.tensor.matmul(ps[:], nff[:, j * 128:(j + 1) * 128],
                                     wt["w1blo"][:], start=True, stop=False)
                    nc.tensor.matmul(ps[:], nff[:, TS2 + j * 128:TS2 + (j + 1) * 128],
                                     wt["w1bhi"][:], start=False, stop=True)
                    nc.any.tensor_copy(stg[:, jj, :], ps[:])
                r0 = g * 512
                nc.sync.dma_start(
                    T2p[r0:r0 + 512, :].rearrange("(j p) c -> p j c", j=4),
                    stg[:])

        # ---------- Phase B: velocity MLP -> vscale [128, NBLK] ----------
        with tc.tile_pool(name="pb", bufs=2) as pb, \
             tc.tile_pool(name="pbp", bufs=2, space="PSUM") as pbp:
            tiles = [(j * 512, 512) for j in range(NPAD // 512)]
            if NPAD % 512:
                tiles.append((NPAD // 512 * 512, NPAD % 512))
            for (o, L) in tiles:
                vps = pbp.tile([64, L], f32, name=f"vps{o}", tag="vps")
                nc.tensor.matmul(vps[:], wt["wv1"][:], nfT_loc[:, o:o + L])
                vh = pb.tile([64, L], f32, name=f"vh{o}", tag="vh")
                nc.scalar.activation(vh[:], vps[:], AF_SILU, bias=wt["bv1c"][:])
                sps = pbp.tile([1, L], f32, name=f"sps{o}", tag="sps")
                nc.tensor.matmul(sps[:], wt["wv2"][:], vh[:])
                vsc = pb.tile([1, L], f32, name=f"vsc{o}", tag="vsc")
                nc.scalar.activation(vsc[:], sps[:], AF.Identity, bias=wt["bv2c"][:])
                for k in range(L // 128):
                    tp = pbp.tile([128, 1], f32, name=f"tp{o}_{k}", tag="tp")
                    nc.tensor.transpose(tp[:], vsc[:, k * 128:(k + 1) * 128],
                                        wt["eye64"][0:1, 0:1])
                    nc.vector.tensor_copy(vscale[:, o // 128 + k:o // 128 + k + 1], tp[:])

        # ---------- Edge sweep (phase C fused per block) ----------
        if STAGE >= 2:
            _edge_sweep(nc, tc, STAGE, wt, g2i, ohTg_d, ohtS_d,
                        cdem_d, efcdn_d, T2p, nfT_loc, aggsb,
                        node_nm, invcnt, vscale, out_d)

    nc.compile()
    return nc


def _edge_sweep(nc, tc, STAGE, wt, g2i, ohTg_d, ohtS_d,
                cdem_d, efcdn_d, T2p, nfT_loc, aggsb,
                node_nm, invcnt, vscale, out_d):
    with tc.tile_pool(name="pgg", bufs=4) as pgg, \
         tc.tile_pool(name="pg", bufs=3) as pg, \
         tc.tile_pool(name="pe", bufs=3) as pe, \
         tc.tile_pool(name="pch", bufs=6) as pch, \
         tc.tile_pool(name="px1", bufs=2, space="PSUM") as px1, \
         tc.tile_pool(name="pmc", bufs=3, space="PSUM") as pmc, \
         tc.tile_pool(name="pst", bufs=2, space="PSUM") as pst, \
         tc.tile_pool(name="pagg", bufs=1, space="PSUM") as pagg:

        def phase_c(b):
            cols = slice(b * 128, (b + 1) * 128)
            xnT = pch.tile([128, 128], f32, name=f"xnT{b}", tag="xnT")
            nc.vector.tensor_copy(xnT[0:64, :], nfT_loc[:, cols])
            nc.vector.tensor_copy(xnT[64:128, :], aggsb[64:128, cols])
            n1 = pagg.tile([64, 128], f32, name=f"n1{b}", tag="aggT")
            nc.tensor.matmul(n1[:], wt["wn1"][:], xnT[:])
            hn = pch.tile([64, 128], f32, name=f"hn{b}", tag="hn")
            nc.scalar.activation(hn[:], n1[:], AF_SILU, bias=wt["bn1c"][:])
            n2 = pagg.tile([64, 128], f32, name=f"n2{b}", tag="aggT")
            nc.tensor.matmul(n2[:], wt["wn2"][:], hn[:])
            hn2 = pch.tile([64, 128], f32, name=f"hn2{b}", tag="hn2")
            nc.scalar.activation(hn2[:], n2[:], AF.Identity, bias=wt["bn2c"][:])
            nmb = node_nm[:, b * 70:(b + 1) * 70]
            ndel = pagg.tile([128, 64], f32, name=f"ndel{b}", tag="aggT")
            nc.tensor.transpose(ndel[:], hn2[:], wt["eye64"][:])
            ot67 = pch.tile([128, 64], f32, name=f"ot67{b}", tag="ot67")
            nc.vector.tensor_tensor(ot67[:], nmb[:, 6:70], ndel[:], OP.add)
            ctp = pagg.tile([128, 3], f32, name=f"ctp{b}", tag="aggT")
            nc.tensor.transpose(ctp[:], aggsb[0:3, cols], wt["eye64"][0:3, 0:3])
            otc = pch.tile([128, 6], f32, name=f"otc{b}", tag="otc")
            t1 = pch.tile([128, 3], f32, name=f"t1{b}", tag="t1")
            nc.scalar.activation(t1[:], ctp[:], AF.Identity,
                                 scale=invcnt[:, b:b + 1])
            t2 = pch.tile([128, 3], f32, name=f"t2{b}", tag="t2")
            nc.scalar.activation(t2[:], nmb[:, 3:6], AF.Identity,
                                 scale=vscale[:, b:b + 1])
            nc.scalar.activation(otc[:, 3:6], nmb[:, 3:6], AF.Identity)
            t3 = pch.tile([128, 3], f32, name=f"t3{b}", tag="t3")
            nc.vector.tensor_tensor(t3[:], t1[:], t2[:], OP.add)
            nc.vector.tensor_tensor(otc[:, 0:3], t3[:], nmb[:, 0:3], OP.add)
            nc.sync.dma_start(out_d[b * 128:(b + 1) * 128, 0:6], otc[:])
            nc.sync.dma_start(out_d[b * 128:(b + 1) * 128, 6:70], ot67[:])

        for b in range(NBLK):
            g2x = pgg.tile([128, 192], i16, name=f"g2x{b}", tag="g2x")
            nc.sync.dma_start(g2x[:], g2i[b])
            g2t = pgg.tile([128, BLKE], bf16, name=f"g2t{b}", tag="g2t")
            nc.gpsimd.dma_gather(
                out_ap=g2t[:].rearrange("p (o n) -> p o n", o=1),
                in_ap=T2p[:], idxs_ap=g2x[:],
                num_idxs=BLKE, num_idxs_reg=BLKE, elem_size=128, transpose=True,
                single_packet=SINGLE_PACKET)
            ohT = pg.tile([128, BLKE], bf16, name=f"ohT{b}", tag="ohT")
            nc.sync.dma_start(ohT[:], ohTg_d[b])
            ohts = pg.tile([128, 24, 128], bf16, name=f"ohts{b}", tag="ohts")
            nc.sync.dma_start(ohts[:], ohtS_d[b])
            cdt = pg.tile([128, 24, 3], f32, name=f"cdt{b}", tag="cdt")
            nc.sync.dma_start(cdt[:], cdem_d[b])
            eftb = pg.tile([34, NSUP * 512], bf16, name=f"eftb{b}", tag="eftb")
            nc.sync.dma_start(eftb[:], efcdn_d[b])

            # start-side first-layer partials for this block's 128 nodes
            p1ps = px1.tile([128, 64], f32, name=f"p1ps{b}", tag="x1")
            nc.tensor.matmul(p1ps[:], nfT_loc[:, b * 128:(b + 1) * 128],
                             wt["w1a"][:])
            p1b = pg.tile([128, 64], bf16, name=f"p1b{b}", tag="p1b")
            nc.scalar.activation(p1b[:], p1ps[:], AF.Identity)

            if STAGE == 2:
                nc.any.tensor_copy(aggsb[:, b * 128:(b + 1) * 128],
                                   g2t[:, 0:128])
                continue

            # ---- stage 1: x1 accumulation + first silu (per supertile) ----
            h1s = []
            for s in range(NSUP):
                sl = slice(s * 512, (s + 1) * 512)
                slh = slice(CELL + s * 512, CELL + (s + 1) * 512)
                x1 = px1.tile([128, 512], f32, name=f"x1{b}_{s}", tag="x1")
                nc.tensor.matmul(x1[0:64, :], wt["injA"][:], g2t[:, sl],
                                 start=True, stop=False, skip_group_check=True)
                nc.tensor.matmul(x1[64:128, :], wt["injB"][:], g2t[:, slh],
                                 start=True, stop=False,
                                 tile_position=(0, 64), skip_group_check=True)
                nc.tensor.matmul(x1[0:64, :], p1b[:],
                                 ohT[:, sl], start=False, stop=False,
                                 skip_group_check=True)
                nc.tensor.matmul(x1[64:128, :], p1b[:], ohT[:, slh],
                                 start=False, stop=False,
                                 tile_position=(0, 64), skip_group_check=True)
                nc.tensor.matmul(x1[:], wt["wefcdn"][:],
                                 eftb[:, s * 512:(s + 1) * 512],
                                 start=False, stop=True, skip_group_check=True)
                h1 = pe.tile([128, 512], bf16, name=f"h1{b}_{s}", tag="h1")
                nc.scalar.activation(h1[:], x1[:], AF_SILU, bias=wt["be1s"][:])
                h1s.append(h1)

            # previous block's node update fills engine bubbles here
            if STAGE >= 5 and b > 0:
                phase_c(b - 1)

            # ---- stage 2: message matmul + silu ----
            msgs = []
            for s in range(NSUP):
                mp = pmc.tile([128, 512], f32, name=f"mp{b}_{s}", tag="mmid")
                nc.tensor.matmul(mp[:], wt["wde2"][:], h1s[s][:])
                msgT = pe.tile([128, 512], bf16, name=f"msgT{b}_{s}", tag="msgT")
                nc.scalar.activation(msgT[:], mp[:], AF_SILU, bias=wt["be2s"][:])
                msgs.append(msgT)

            # ---- stage 3: coord-hidden matmul; build mcA=[msgA;chA], mcB=[chB;msgB]
            mcs = []
            for s in range(NSUP):
                cpx = pmc.tile([128, 512], f32, name=f"cp{b}_{s}", tag="mmid")
                nc.tensor.matmul(cpx[:], wt["wdc1x"][:], msgs[s][:])
                mcA = pe.tile([128, 512], bf16, name=f"mcA{b}_{s}", tag="mcA")
                mcB = pe.tile([128, 512], bf16, name=f"mcB{b}_{s}", tag="mcB")
                nc.scalar.activation(mcA[0:64, :], msgs[s][0:64, :], AF.Identity)
                nc.scalar.activation(mcB[64:128, :], msgs[s][64:128, :], AF.Identity)
                nc.scalar.activation(mcA[64:128, :], cpx[64:128, :], AF_SILU,
                                     bias=wt["bc1s"][64:128, :])
                nc.scalar.activation(mcB[0:64, :], cpx[0:64, :], AF_SILU,
                                     bias=wt["bc1s"][0:64, :])
                mcs.append((mcA, mcB))

            if STAGE == 3:
                nc.any.tensor_copy(aggsb[:, b * 128:(b + 1) * 128],
                                   mcs[0][0][:, 0:128])
                continue

            # ---- stage 4: edge-major flip (msg|gate|coord) + gate + scatter src
            # One merged scatter-source tile per block: chunk jb cols 0:3 =
            # coord term -> agg rows 0:3; cols 64:128 = msg*gate -> agg rows
            # 64:128; cols 3:64 land in unread agg rows (zeroed once).
            rcg = pch.tile([128, 24, 128], bf16, name=f"rcg{b}", tag="rcg")
            nc.vector.memset(rcg[:, :, 3:64], 0)
            for s in range(NSUP):
                for g in range(2):
                    mc = mcs[s][g]
                    rawc = wt["rawcA"] if g == 0 else wt["rawcB"]
                    jb = 12 * g + s * 4
                    st = pst.tile([128, 4, 66], f32, name=f"st{b}_{s}_{g}", tag="st")
                    for c4 in range(4):
                        cc = slice(c4 * 128, (c4 + 1) * 128)
                        nc.tensor.matmul(st[:, c4, :], mc[:, cc], rawc[:],
                                         start=True, stop=True)
                    tnh = pch.tile([128, 4], f32, name=f"tnh{b}_{s}_{g}", tag="tnh")
                    nc.scalar.activation(tnh[:], st[:, :, 64:65].squeeze(2),
                                         AF.Tanh, bias=wt["bih"][:], scale=0.5)
                    gate = pch.tile([128, 4], f32, name=f"gt{b}_{s}_{g}", tag="gate")
                    nc.vector.tensor_scalar(out=gate[:], in0=tnh[:], scalar1=1.0,
                                            scalar2=0.5, op0=OP.add, op1=OP.mult)
                    nc.vector.tensor_tensor(
                        rcg[:, jb:jb + 4, 64:128], st[:, :, 0:64],
                        gate[:].unsqueeze(2).broadcast_to([128, 4, 64]), OP.mult)
                    nc.vector.tensor_tensor(
                        rcg[:, jb:jb + 4, 0:3], cdt[:, jb:jb + 4, :],
                        st[:, :, 65:66].broadcast_to([128, 4, 3]), OP.mult)

            # ---- stage 5: segment-sum scatter into aggT ----
            aggT = pagg.tile([128, 128], f32, name=f"aggT{b}", tag="aggT")
            for j in range(24):
                nc.tensor.matmul(aggT[:, :], rcg[:, j, :],
                                 ohts[:, j, :],
                                 start=(j == 0), stop=(j == 23),
                                 tile_position=(0, 0), skip_group_check=True)
            nc.scalar.activation(aggsb[:, b * 128:(b + 1) * 128], aggT[:], AF.Identity)

        if STAGE >= 5:
            phase_c(NBLK - 1)


def kernel(**inputs):
    ei = np.asarray(inputs["edge_indices"])
    start = ei[0].astype(np.int64)
    end = ei[1].astype(np.int64)
    ef = _f(inputs["edge_features"])
    nfi = _f(inputs["node_features_input"])
    coords = nfi[:, 0:3]
    cd_all = coords[start] - coords[end]
    cdn_all = np.sqrt((cd_all ** 2).sum(1)).astype(np.float32)
    deg = np.bincount(start, minlength=N).astype(np.float32)
    invcnt_all = (1.0 / np.maximum(deg, 1.0)).astype(np.float32)

    w = _prep_weights(inputs)
    nfT_full = np.zeros((64, NFULL), np.float32)
    nfT_full[:, 0:N] = nfi[:, 6:70].T
    nfT_full = nfT_full.astype(mybir.dt.np(bf16))

    in_maps = []
    for c in range(NCORES):
        d = _prep_core(c, start, end, ef, nfi, cd_all, cdn_all, invcnt_all)
        d.update(w)
        d["nfT_full"] = nfT_full
        in_maps.append(d)

    if "nc" not in _cache:
        _cache["nc"] = _build_program()
    nc = _cache["nc"]
    _cache["in_maps"] = in_maps
    res = run_bass_kernel_spmd(nc, in_maps, list(range(NCORES)))
    out = np.empty((N, 70), np.float32)
    for c in range(NCORES):
        out[c * NPC:(c + 1) * NPC] = res.results[c]["out"][0:NPC]
    return out



# revision 4
# speedup vs baseline: 1.4976x; 1.4976x over previous
"""EquivariantGraphConvolution (EGNN layer) on 8 Trainium2 NeuronCores.

Strategy (v3)
-------------
Nodes are range-partitioned across the 8 cores (6250 nodes each); every edge is
owned by the core that owns its *start* node, so the per-start segment sums are
core-local and no collective is needed.  Per core, edges are bucketed by
128-node start block and padded to a static layout of 49 blocks x 24 chunks x
128 edges (3072 slots/block).

v2 fetched end-node partials with a dma_gather per block; the trace showed the
49 gathers cost 28us each of serial GpSimd (Q7 descriptor generation) -- 1.38ms
of the 1.91ms span.  v3 removes all gathers and all host-materialized one-hot
matrices from HBM:
  * The host materializes [nf[start] ; nf[end]] per edge slot as a single
    [128, 3072] bf16 stream per block (128B/edge).  Stage 1 is then just
    2 matmuls with the [W1a; W1b] stationary (+1 for ef/coord-norm terms).
  * The scatter one-hot is built on-chip with one DVE is_equal per block from
    a [128, 24] lid stream (edge-slot-major) against an arange constant.
  * The segment-sum uses rcg [128 edges, 67] as the matmul stationary and the
    one-hot as moving operand, accumulating aggT [67 feats, 128 nodes].
"""
import sys
sys.path.insert(0, "/opt/trn_rl_repo")
import contextlib
import os
import numpy as np

import concourse.bass as bass
import concourse.bacc as bacc
import concourse.mybir as mybir
import concourse.tile as tile
from concourse.bass_utils import run_bass_kernel_spmd

f32 = mybir.dt.float32
bf16 = mybir.dt.bfloat16
i16 = mybir.dt.int16
i32 = mybir.dt.int32
AF = mybir.ActivationFunctionType
OP = mybir.AluOpType

# ---- problem constants (hardcoded per contract) ----
N = 50000
E = 1_000_000
H = 64
EF = 16
NCORES = 8
NPC = N // NCORES          # 6250 nodes per core
NBLK = 49                  # 128-node blocks per core (49*128 = 6272 >= 6250)
NPAD = NBLK * 128          # 6272
NCH = 24                   # 128-edge chunks per block
BLKE = NCH * 128           # 3072 edge slots per block
ES = NBLK * BLKE           # 150528 edge slots per core
NSUP = BLKE // 1024        # supertile pairs: 3 supertiles of 512 per half
GRP = 6                    # chunks per flip/gate group (24 = 4 * 6)

# CoreSim does not implement Silu; substitute Tanh for structural sim checks.
SIM_ACT = os.environ.get("EGNN_SIMACT", "0") == "1"
AF_SILU = AF.Tanh if SIM_ACT else AF.Silu
AF_SIGM = AF.Tanh if SIM_ACT else AF.Sigmoid

_cache = {}


def _f(x):
    return np.ascontiguousarray(x, np.float32)


def _prep_weights(inp):
    """Small weight/constant tensors, identical on all cores."""
    W_e1 = _f(inp["W_e1"])           # [145, 64]
    w = {}
    w["wse"] = W_e1[0:128].copy()                    # [128, 64] start|end rows
    wef = np.zeros((34, 128), np.float32)
    wef[0:16, 0:64] = W_e1[129:145]
    wef[16:17, 0:64] = W_e1[128:129]
    wef[17:33, 64:128] = W_e1[129:145]
    wef[33:34, 64:128] = W_e1[128:129]
    w["wefcdn"] = wef
    W_e2 = _f(inp["W_e2"]); W_c1 = _f(inp["W_c1"])
    bd = np.zeros((128, 128), np.float32)
    bd[0:64, 0:64] = W_e2; bd[64:128, 64:128] = W_e2
    w["wde2"] = bd
    # swapped block-diagonal: out rows 0:64 = W_c1^T msgB, rows 64:128 = W_c1^T msgA
    bcx = np.zeros((128, 128), np.float32)
    bcx[64:128, 0:64] = W_c1
    bcx[0:64, 64:128] = W_c1
    w["wdc1x"] = bcx
    W_i = _f(inp["W_i"]); W_c2 = _f(inp["W_c2"])
    # mcA = [msgA ; chA]  -> out cols: 0:64 msg, 64 gate logit, 65 coord weight
    ra = np.zeros((128, 66), np.float32)
    ra[0:64, 0:64] = np.eye(64); ra[0:64, 64:65] = W_i; ra[64:128, 65:66] = W_c2
    w["rawcA"] = ra
    # mcB = [chB ; msgB]
    rb = np.zeros((128, 66), np.float32)
    rb[64:128, 0:64] = np.eye(64); rb[64:128, 64:65] = W_i; rb[0:64, 65:66] = W_c2
    w["rawcB"] = rb
    w["wn1"] = _f(inp["W_n1"])                        # [128, 64]
    w["wn2"] = _f(inp["W_n2"])
    w["wv1"] = _f(inp["W_v1"])
    w["wv2"] = _f(inp["W_v2"])
    w["eye64"] = np.eye(64, dtype=np.float32)
    w["arange128"] = np.tile(np.arange(128, dtype=np.float32), (128, 1))

    b_e1 = _f(inp["b_e1"]); b_e2 = _f(inp["b_e2"]); b_c1 = _f(inp["b_c1"])
    w["be1s"] = np.concatenate([b_e1, b_e1]).reshape(128, 1)
    w["be2s"] = np.concatenate([b_e2, b_e2]).reshape(128, 1)
    w["bc1s"] = np.concatenate([b_c1, b_c1]).reshape(128, 1)
    w["bih"] = np.full((128, 1), float(np.asarray(inp["b_i"]).ravel()[0]), np.float32)
    w["bn1c"] = _f(inp["b_n1"]).reshape(64, 1)
    w["bn2c"] = _f(inp["b_n2"]).reshape(64, 1)
    w["bv1c"] = _f(inp["b_v1"]).reshape(64, 1)
    w["bv2c"] = np.full((1, 1), float(np.asarray(inp["b_v2"]).ravel()[0]), np.float32)
    return w


def _prep_core(c, start, end, ef, nfi, nf_bf, cd_all, cdn_all, invcnt_all):
    bfdt = mybir.dt.np(bf16)
    lo, hi = c * NPC, (c + 1) * NPC
    sel = (start >= lo) & (start < hi)
    eo = np.nonzero(sel)[0]
    s_loc = (start[eo] - lo).astype(np.int64)
    blk = s_loc >> 7
    order = np.argsort(blk, kind="stable")
    eo = eo[order]; s_loc = s_loc[order]; blk = blk[order]
    counts = np.bincount(blk, minlength=NBLK)
    if counts.max() > BLKE:
        raise RuntimeError(f"block overflow: {counts.max()} > {BLKE}")
    starts_ = np.zeros(NBLK, np.int64)
    starts_[1:] = np.cumsum(counts)[:-1]
    within = np.arange(len(eo)) - starts_[blk]
    slots = blk * BLKE + within

    sg = np.zeros(ES, np.int64)          # global start per slot (pad -> node lo)
    sg[:] = lo
    eg = np.zeros(ES, np.int64)          # global end per slot (pad -> node 0)
    lid = np.full(ES, -1.0, np.float32)
    cds = np.zeros((ES, 3), np.float32)
    cdns = np.zeros(ES, np.float32)
    efs = np.zeros((ES, EF), np.float32)
    sg[slots] = start[eo]
    eg[slots] = end[eo]
    lid[slots] = (s_loc & 127).astype(np.float32)
    cds[slots] = cd_all[eo]
    cdns[slots] = cdn_all[eo]
    efs[slots] = ef[eo]

    d = {}
    # [NBLK, 128, BLKE] bf16: rows 0:64 = nf[start].T, 64:128 = nf[end].T
    nfse = np.empty((ES, 128), bfdt)
    nfse[:, 0:64] = nf_bf[sg]
    nfse[:, 64:128] = nf_bf[eg]
    d["nfse"] = np.ascontiguousarray(
        nfse.reshape(NBLK, BLKE, 128).transpose(0, 2, 1))
    # scatter-orientation lid stream [NBLK, 128, NCH] (edge-in-chunk, chunk)
    lidc = lid.reshape(NBLK, NCH, 128)
    d["lidc"] = np.ascontiguousarray(lidc.transpose(0, 2, 1)).astype(bfdt)
    d["cdem"] = cds.reshape(NBLK, NCH, 128, 3).transpose(0, 2, 1, 3).copy()  # [NBLK,128,24,3]
    efr = efs.reshape(NBLK, 2, NSUP, 512, EF)
    cdnr = cdns.reshape(NBLK, 2, NSUP, 512)
    eA = np.concatenate([efr[:, 0].transpose(0, 1, 3, 2),
                         cdnr[:, 0][:, :, None, :]], axis=2)             # [NBLK,3,17,512]
    eB = np.concatenate([efr[:, 1].transpose(0, 1, 3, 2),
                         cdnr[:, 1][:, :, None, :]], axis=2)
    ecat = np.concatenate([eA, eB], axis=2)                              # [NBLK,3,34,512]
    d["efcdn"] = np.ascontiguousarray(
        ecat.transpose(0, 2, 1, 3).reshape(NBLK, 34, NSUP * 512)).astype(bfdt)

    nm = np.zeros((NPAD, 70), np.float32)
    nm[0:NPC] = nfi[lo:hi]
    d["node_nm"] = nm.reshape(NBLK, 128, 70).transpose(1, 0, 2).reshape(128, NBLK * 70).copy()
    ic = np.ones(NPAD, np.float32)
    ic[0:NPC] = invcnt_all[lo:hi]
    d["invcnt"] = ic.reshape(NBLK, 128).T.copy()                         # [128, NBLK]
    nl = np.zeros((64, NPAD), np.float32)
    nl[:, 0:NPC] = nfi[lo:hi, 6:70].T
    d["nfT_local"] = nl
    return d


def _build_program():
    STAGE = int(os.environ.get("EGNN_STAGE", "5"))
    nc = bacc.Bacc("TRN2", target_bir_lowering=False, debug=False,
                   enable_asserts=False, num_devices=NCORES)

    def din(name, shape, dt=f32):
        return nc.dram_tensor(name, list(shape), dt, kind="ExternalInput").ap()

    nfse_d = din("nfse", [NBLK, 128, BLKE], bf16)
    lidc_d = din("lidc", [NBLK, 128, NCH], bf16)
    cdem_d = din("cdem", [NBLK, 128, NCH, 3])
    efcdn_d = din("efcdn", [NBLK, 34, NSUP * 512], bf16)
    invcnt_d = din("invcnt", [128, NBLK])
    node_nm_d = din("node_nm", [128, NBLK * 70])
    nfT_loc_d = din("nfT_local", [64, NPAD])
    wnames = ["wse", "wefcdn", "wde2", "wdc1x", "rawcA", "rawcB",
              "wn1", "wn2", "wv1", "wv2", "eye64", "arange128",
              "be1s", "be2s", "bc1s", "bih", "bn1c", "bn2c", "bv1c", "bv2c"]
    wshapes = {"wse": [128, 64], "wefcdn": [34, 128],
               "wde2": [128, 128], "wdc1x": [128, 128],
               "rawcA": [128, 66], "rawcB": [128, 66],
               "wn1": [128, 64], "wn2": [64, 64],
               "wv1": [64, 64], "wv2": [64, 1], "eye64": [64, 64],
               "arange128": [128, 128],
               "be1s": [128, 1], "be2s": [128, 1], "bc1s": [128, 1],
               "bih": [128, 1], "bn1c": [64, 1], "bn2c": [64, 1],
               "bv1c": [64, 1], "bv2c": [1, 1]}
    wd = {n: din(n, wshapes[n]) for n in wnames}
    out_d = nc.dram_tensor("out", [NPAD, 70], f32, kind="ExternalOutput").ap()

    # weights that are matmul operands in the bf16 pipeline
    BF_W = ("wse", "wefcdn", "wde2", "wdc1x", "rawcA", "rawcB", "arange128")

    with tile.TileContext(nc) as tc, contextlib.ExitStack() as ctx:
        wpool = ctx.enter_context(tc.tile_pool(name="w", bufs=1))
        wt = {}
        for n in wnames:
            dt = bf16 if n in BF_W else f32
            t = wpool.tile(wshapes[n], dt, name=f"wt_{n}")
            if dt == f32:
                nc.sync.dma_start(t[:], wd[n][:])
            else:
                tf = wpool.tile(wshapes[n], f32, name=f"wtf_{n}")
                nc.sync.dma_start(tf[:], wd[n][:])
                nc.vector.tensor_copy(t[:], tf[:])
            wt[n] = t
        node_nm = wpool.tile([128, NBLK * 70], f32, name="node_nm")
        nc.sync.dma_start(node_nm[:], node_nm_d[:])
        invcnt = wpool.tile([128, NBLK], f32, name="invcnt")
        nc.sync.dma_start(invcnt[:], invcnt_d[:])
        nfT_loc = wpool.tile([64, NPAD], f32, name="nfT_loc")
        nc.sync.dma_start(nfT_loc[:], nfT_loc_d[:])
        vscale = wpool.tile([128, NBLK], f32, name="vscale")
        aggm = wpool.tile([64, NPAD], f32, name="aggm")
        aggc = wpool.tile([3, NPAD], f32, name="aggc")

        # ---------- Phase B: velocity MLP -> vscale [128, NBLK] ----------
        with tc.tile_pool(name="pb", bufs=2) as pb, \
             tc.tile_pool(name="pbp", bufs=2, space="PSUM") as pbp:
            tiles = [(j * 512, 512) for j in range(NPAD // 512)]
            if NPAD % 512:
                tiles.append((NPAD // 512 * 512, NPAD % 512))
            for (o, L) in tiles:
                vps = pbp.tile([64, L], f32, name=f"vps{o}", tag="vps")
                nc.tensor.matmul(vps[:], wt["wv1"][:], nfT_loc[:, o:o + L])
                vh = pb.tile([64, L], f32, name=f"vh{o}", tag="vh")
                nc.scalar.activation(vh[:], vps[:], AF_SILU, bias=wt["bv1c"][:])
                sps = pbp.tile([1, L], f32, name=f"sps{o}", tag="sps")
                nc.tensor.matmul(sps[:], wt["wv2"][:], vh[:])
                vsc = pb.tile([1, L], f32, name=f"vsc{o}", tag="vsc")
                nc.scalar.activation(vsc[:], sps[:], AF.Identity, bias=wt["bv2c"][:])
                for k in range(L // 128):
                    tp = pbp.tile([128, 1], f32, name=f"tp{o}_{k}", tag="tp")
                    nc.tensor.transpose(tp[:], vsc[:, k * 128:(k + 1) * 128],
                                        wt["eye64"][0:1, 0:1])
                    nc.vector.tensor_copy(vscale[:, o // 128 + k:o // 128 + k + 1], tp[:])

        # ---------- Edge sweep (phase C fused per block) ----------
        if STAGE >= 2:
            _edge_sweep(nc, tc, STAGE, wt, nfse_d, lidc_d,
                        cdem_d, efcdn_d, nfT_loc, aggm, aggc,
                        node_nm, invcnt, vscale, out_d)

    nc.compile()
    return nc


def _edge_sweep(nc, tc, STAGE, wt, nfse_d, lidc_d,
                cdem_d, efcdn_d, nfT_loc, aggm, aggc,
                node_nm, invcnt, vscale, out_d):
    with tc.tile_pool(name="pg", bufs=3) as pg, \
         tc.tile_pool(name="pe", bufs=3) as pe, \
         tc.tile_pool(name="pch", bufs=6) as pch, \
         tc.tile_pool(name="poh", bufs=2) as poh, \
         tc.tile_pool(name="px1", bufs=2, space="PSUM") as px1, \
         tc.tile_pool(name="pmc", bufs=3, space="PSUM") as pmc, \
         tc.tile_pool(name="pst", bufs=2, space="PSUM") as pst, \
         tc.tile_pool(name="pagg", bufs=1, space="PSUM") as pagg:

        def phase_c(b):
            cols = slice(b * 128, (b + 1) * 128)
            xnT = pch.tile([128, 128], f32, name=f"xnT{b}", tag="xnT")
            nc.vector.tensor_copy(xnT[0:64, :], nfT_loc[:, cols])
            nc.vector.tensor_copy(xnT[64:128, :], aggm[:, cols])
            n1 = pagg.tile([64, 128], f32, name=f"n1{b}", tag="aggT")
            nc.tensor.matmul(n1[:], wt["wn1"][:], xnT[:])
            hn = pch.tile([64, 128], f32, name=f"hn{b}", tag="hn")
            nc.scalar.activation(hn[:], n1[:], AF_SILU, bias=wt["bn1c"][:])
            n2 = pagg.tile([64, 128], f32, name=f"n2{b}", tag="aggT")
            nc.tensor.matmul(n2[:], wt["wn2"][:], hn[:])
            hn2 = pch.tile([64, 128], f32, name=f"hn2{b}", tag="hn2")
            nc.scalar.activation(hn2[:], n2[:], AF.Identity, bias=wt["bn2c"][:])
            nmb = node_nm[:, b * 70:(b + 1) * 70]
            ndel = pagg.tile([128, 64], f32, name=f"ndel{b}", tag="aggT")
            nc.tensor.transpose(ndel[:], hn2[:], wt["eye64"][:])
            ot67 = pch.tile([128, 64], f32, name=f"ot67{b}", tag="ot67")
            nc.vector.tensor_tensor(ot67[:], nmb[:, 6:70], ndel[:], OP.add)
            ctp = pagg.tile([128, 3], f32, name=f"ctp{b}", tag="aggT")
            nc.tensor.transpose(ctp[:], aggc[:, cols], wt["eye64"][0:3, 0:3])
            otc = pch.tile([128, 6], f32, name=f"otc{b}", tag="otc")
            t1 = pch.tile([128, 3], f32, name=f"t1{b}", tag="t1")
            nc.scalar.activation(t1[:], ctp[:], AF.Identity,
                                 scale=invcnt[:, b:b + 1])
            t2 = pch.tile([128, 3], f32, name=f"t2{b}", tag="t2")
            nc.scalar.activation(t2[:], nmb[:, 3:6], AF.Identity,
                                 scale=vscale[:, b:b + 1])
            nc.scalar.activation(otc[:, 3:6], nmb[:, 3:6], AF.Identity)
            t3 = pch.tile([128, 3], f32, name=f"t3{b}", tag="t3")
            nc.vector.tensor_tensor(t3[:], t1[:], t2[:], OP.add)
            nc.vector.tensor_tensor(otc[:, 0:3], t3[:], nmb[:, 0:3], OP.add)
            nc.sync.dma_start(out_d[b * 128:(b + 1) * 128, 0:6], otc[:])
            nc.sync.dma_start(out_d[b * 128:(b + 1) * 128, 6:70], ot67[:])

        for b in range(NBLK):
            nfse = pg.tile([128, BLKE], bf16, name=f"nfse{b}", tag="nfse")
            nc.sync.dma_start(nfse[:], nfse_d[b])
            lidt = pg.tile([128, NCH], bf16, name=f"lidt{b}", tag="lidt")
            nc.sync.dma_start(lidt[:], lidc_d[b])
            cdt = pg.tile([128, NCH, 3], f32, name=f"cdt{b}", tag="cdt")
            nc.sync.dma_start(cdt[:], cdem_d[b])
            eftb = pg.tile([34, NSUP * 512], bf16, name=f"eftb{b}", tag="eftb")
            nc.sync.dma_start(eftb[:], efcdn_d[b])

            # scatter one-hot [128 edge-in-chunk, 24 chunk, 128 node] on-chip
            ohts = poh.tile([128, NCH, 128], bf16, name=f"ohts{b}", tag="ohts")
            nc.vector.tensor_tensor(
                ohts[:],
                lidt[:].unsqueeze(2).broadcast_to([128, NCH, 128]),
                wt["arange128"][:].unsqueeze(1).broadcast_to([128, NCH, 128]),
                OP.is_equal)

            if STAGE == 2:
                nc.any.tensor_copy(aggm[:, b * 128:(b + 1) * 128],
                                   ohts[0:64, 0, :])
                continue

            # ---- stage 1: x1 accumulation + first silu (per supertile) ----
            h1s = []
            for s in range(NSUP):
                sl = slice(s * 512, (s + 1) * 512)
                slh = slice(NSUP * 512 + s * 512, NSUP * 512 + (s + 1) * 512)
                x1 = px1.tile([128, 512], f32, name=f"x1{b}_{s}", tag="x1")
                nc.tensor.matmul(x1[0:64, :], wt["wse"][:], nfse[:, sl],
                                 start=True, stop=False, skip_group_check=True)
                nc.tensor.matmul(x1[64:128, :], wt["wse"][:], nfse[:, slh],
                                 start=True, stop=False,
                                 tile_position=(0, 64), skip_group_check=True)
                nc.tensor.matmul(x1[:], wt["wefcdn"][:],
                                 eftb[:, s * 512:(s + 1) * 512],
                                 start=False, stop=True, skip_group_check=True)
                h1 = pe.tile([128, 512], bf16, name=f"h1{b}_{s}", tag="h1")
                nc.scalar.activation(h1[:], x1[:], AF_SILU, bias=wt["be1s"][:])
                h1s.append(h1)

            # previous block's node update fills engine bubbles here
            if STAGE >= 5 and b > 0:
                phase_c(b - 1)

            # ---- stage 2: message matmul + silu ----
            msgs = []
            for s in range(NSUP):
                mp = pmc.tile([128, 512], f32, name=f"mp{b}_{s}", tag="mmid")
                nc.tensor.matmul(mp[:], wt["wde2"][:], h1s[s][:])
                msgT = pe.tile([128, 512], bf16, name=f"msgT{b}_{s}", tag="msgT")
                nc.scalar.activation(msgT[:], mp[:], AF_SILU, bias=wt["be2s"][:])
                msgs.append(msgT)

            # ---- stage 3: coord-hidden matmul; build mcA=[msgA;chA], mcB=[chB;msgB]
            mcs = []
            for s in range(NSUP):
                cpx = pmc.tile([128, 512], f32, name=f"cp{b}_{s}", tag="mmid")
                nc.tensor.matmul(cpx[:], wt["wdc1x"][:], msgs[s][:])
                mcA = pe.tile([128, 512], bf16, name=f"mcA{b}_{s}", tag="mcA")
                mcB = pe.tile([128, 512], bf16, name=f"mcB{b}_{s}", tag="mcB")
                nc.scalar.activation(mcA[0:64, :], msgs[s][0:64, :], AF.Identity)
                nc.scalar.activation(mcB[64:128, :], msgs[s][64:128, :], AF.Identity)
                nc.scalar.activation(mcA[64:128, :], cpx[64:128, :], AF_SILU,
                                     bias=wt["bc1s"][64:128, :])
                nc.scalar.activation(mcB[0:64, :], cpx[0:64, :], AF_SILU,
                                     bias=wt["bc1s"][0:64, :])
                mcs.append((mcA, mcB))

            if STAGE == 3:
                nc.any.tensor_copy(aggm[:, b * 128:(b + 1) * 128],
                                   mcs[0][0][0:64, 0:128])
                continue

            # ---- stage 4: edge-major flip (msg|gate|coord) + gate ----
            # rcg [128 edge, 24 chunk, 67]: cols 0:64 msg*gate, 64:67 coord
            rcg = pch.tile([128, NCH, 67], bf16, name=f"rcg{b}", tag="rcg")
            for g4 in range(NCH // GRP):
                st = pst.tile([128, GRP, 66], f32, name=f"st{b}_{g4}", tag="st")
                for cg in range(GRP):
                    j = g4 * GRP + cg                 # chunk 0..23
                    if j < NCH // 2:
                        mc = mcs[j // 4][0]           # A half
                        rawc = wt["rawcA"]
                        cc = slice((j % 4) * 128, (j % 4 + 1) * 128)
                    else:
                        mc = mcs[(j - NCH // 2) // 4][1]
                        rawc = wt["rawcB"]
                        cc = slice(((j - NCH // 2) % 4) * 128,
                                   ((j - NCH // 2) % 4 + 1) * 128)
                    nc.tensor.matmul(st[:, cg, :], mc[:, cc], rawc[:],
                                     start=True, stop=True)
                jb = slice(g4 * GRP, (g4 + 1) * GRP)
                gate = pch.tile([128, GRP], f32, name=f"gt{b}_{g4}", tag="gate")
                nc.scalar.activation(gate[:], st[:, :, 64:65].squeeze(2),
                                     AF_SIGM, bias=wt["bih"][:])
                nc.vector.tensor_tensor(
                    rcg[:, jb, 0:64], st[:, :, 0:64],
                    gate[:].unsqueeze(2).broadcast_to([128, GRP, 64]), OP.mult)
                nc.vector.tensor_tensor(
                    rcg[:, jb, 64:67], cdt[:, jb, :],
                    st[:, :, 65:66].broadcast_to([128, GRP, 3]), OP.mult)

            # ---- stage 5: segment-sum scatter into aggT [67, 128 nodes] ----
            # rows 0:64 msg-sum, 64:67 coord-sum
            aggT = pagg.tile([67, 128], f32, name=f"aggT{b}", tag="aggT")
            for j in range(NCH):
                nc.tensor.matmul(aggT[:, :], rcg[:, j, :],
                                 ohts[:, j, :],
                                 start=(j == 0), stop=(j == NCH - 1))
            nc.scalar.activation(aggm[:, b * 128:(b + 1) * 128], aggT[0:64, :], AF.Identity)
            nc.scalar.activation(aggc[:, b * 128:(b + 1) * 128], aggT[64:67, :], AF.Identity)

        if STAGE >= 5:
            phase_c(NBLK - 1)


def kernel(**inputs):
    ei = np.asarray(inputs["edge_indices"])
    start = ei[0].astype(np.int64)
    end = ei[1].astype(np.int64)
    ef = _f(inputs["edge_features"])
    nfi = _f(inputs["node_features_input"])
    coords = nfi[:, 0:3]
    cd_all = coords[start] - coords[end]
    cdn_all = np.sqrt((cd_all ** 2).sum(1)).astype(np.float32)
    deg = np.bincount(start, minlength=N).astype(np.float32)
    invcnt_all = (1.0 / np.maximum(deg, 1.0)).astype(np.float32)
    nf_bf = nfi[:, 6:70].astype(mybir.dt.np(bf16))

    w = _prep_weights(inputs)

    in_maps = []
    for c in range(NCORES):
        d = _prep_core(c, start, end, ef, nfi, nf_bf, cd_all, cdn_all, invcnt_all)
        d.update(w)
        in_maps.append(d)

    if "nc" not in _cache:
        _cache["nc"] = _build_program()
    nc = _cache["nc"]
    _cache["in_maps"] = in_maps
    res = run_bass_kernel_spmd(nc, in_maps, list(range(NCORES)))
    out = np.empty((N, 70), np.float32)
    for c in range(NCORES):
        out[c * NPC:(c + 1) * NPC] = res.results[c]["out"][0:NPC]
    return out


# revision 8
# speedup vs baseline: 2.1013x; 1.4031x over previous
"""EquivariantGraphConvolution (EGNN layer) on 8 Trainium2 NeuronCores.

Strategy (v3)
-------------
Nodes are range-partitioned across the 8 cores (6250 nodes each); every edge is
owned by the core that owns its *start* node, so the per-start segment sums are
core-local and no collective is needed.  Per core, edges are bucketed by
128-node start block and padded to a static layout of 49 blocks x 24 chunks x
128 edges (3072 slots/block).

v2 fetched end-node partials with a dma_gather per block; the trace showed the
49 gathers cost 28us each of serial GpSimd (Q7 descriptor generation) -- 1.38ms
of the 1.91ms span.  v3 removes all gathers and all host-materialized one-hot
matrices from HBM:
  * The host materializes [nf[start] ; nf[end]] per edge slot as a single
    [128, 3072] bf16 stream per block (128B/edge).  Stage 1 is then just
    2 matmuls with the [W1a; W1b] stationary (+1 for ef/coord-norm terms).
  * The scatter one-hot is built on-chip with one DVE is_equal per block from
    a [128, 24] lid stream (edge-slot-major) against an arange constant.
  * The segment-sum uses rcg [128 edges, 67] as the matmul stationary and the
    one-hot as moving operand, accumulating aggT [67 feats, 128 nodes].
"""
import sys
sys.path.insert(0, "/opt/trn_rl_repo")
import contextlib
import os
import numpy as np

import concourse.bass as bass
import concourse.bacc as bacc
import concourse.mybir as mybir
import concourse.tile as tile
from concourse.bass_utils import run_bass_kernel_spmd

f32 = mybir.dt.float32
bf16 = mybir.dt.bfloat16
i16 = mybir.dt.int16
i32 = mybir.dt.int32
AF = mybir.ActivationFunctionType
OP = mybir.AluOpType

# ---- problem constants (hardcoded per contract) ----
N = 50000
E = 1_000_000
H = 64
EF = 16
NCORES = 8
NPC = N // NCORES          # 6250 nodes per core
NBLK = 49                  # 128-node blocks per core (49*128 = 6272 >= 6250)
NPAD = NBLK * 128          # 6272
NCH = 24                   # 128-edge chunks per block
BLKE = NCH * 128           # 3072 edge slots per block
ES = NBLK * BLKE           # 150528 edge slots per core
NSUP = BLKE // 1024        # supertile pairs: 3 supertiles of 512 per half
GRP = 6                    # chunks per flip/gate group (24 = 4 * 6)

# CoreSim does not implement Silu; substitute Tanh for structural sim checks.
SIM_ACT = os.environ.get("EGNN_SIMACT", "0") == "1"
AF_SILU = AF.Tanh if SIM_ACT else AF.Silu
AF_SIGM = AF.Tanh if SIM_ACT else AF.Sigmoid

_cache = {}


def _f(x):
    return np.ascontiguousarray(x, np.float32)


def _prep_weights(inp):
    """Small weight/constant tensors, identical on all cores."""
    W_e1 = _f(inp["W_e1"])           # [145, 64]
    w = {}
    w["wse"] = W_e1[0:128].copy()                    # [128, 64] start|end rows
    wef = np.zeros((34, 128), np.float32)
    wef[0:16, 0:64] = W_e1[129:145]
    wef[16:17, 0:64] = W_e1[128:129]
    wef[17:33, 64:128] = W_e1[129:145]
    wef[33:34, 64:128] = W_e1[128:129]
    w["wefcdn"] = wef
    W_e2 = _f(inp["W_e2"]); W_c1 = _f(inp["W_c1"])
    bd = np.zeros((128, 128), np.float32)
    bd[0:64, 0:64] = W_e2; bd[64:128, 64:128] = W_e2
    w["wde2"] = bd
    w["wc1d"] = np.concatenate([W_c1, W_c1], axis=0)   # [128, 64], both halves
    W_i = _f(inp["W_i"]); W_c2 = _f(inp["W_c2"])
    # mcA = [msgA ; chA]  -> out cols: 0:64 msg, 64 gate logit, 65 coord weight
    ra = np.zeros((128, 66), np.float32)
    ra[0:64, 0:64] = np.eye(64); ra[0:64, 64:65] = W_i; ra[64:128, 65:66] = W_c2
    w["rawcA"] = ra
    # mcB = [chB ; msgB]
    rb = np.zeros((128, 66), np.float32)
    rb[64:128, 0:64] = np.eye(64); rb[64:128, 64:65] = W_i; rb[0:64, 65:66] = W_c2
    w["rawcB"] = rb
    w["wn1a"] = _f(inp["W_n1"])[0:64].copy()          # [64, 64] nf rows
    w["wn1b"] = _f(inp["W_n1"])[64:128].copy()        # [64, 64] agg rows
    w["wn2"] = _f(inp["W_n2"])
    w["wv1"] = _f(inp["W_v1"])
    w["wv2"] = _f(inp["W_v2"])
    w["eye64"] = np.eye(64, dtype=np.float32)
    w["arange128"] = np.tile(np.arange(128, dtype=np.float32), (128, 1))

    b_e1 = _f(inp["b_e1"]); b_e2 = _f(inp["b_e2"]); b_c1 = _f(inp["b_c1"])
    w["be1s"] = np.concatenate([b_e1, b_e1]).reshape(128, 1)
    w["be2s"] = np.concatenate([b_e2, b_e2]).reshape(128, 1)
    w["bc1s"] = np.concatenate([b_c1, b_c1]).reshape(128, 1)
    w["bih"] = np.full((128, 1), 0.5 * float(np.asarray(inp["b_i"]).ravel()[0]), np.float32)
    w["bn1c"] = _f(inp["b_n1"]).reshape(64, 1)
    w["bn2c"] = _f(inp["b_n2"]).reshape(64, 1)
    w["bv1c"] = _f(inp["b_v1"]).reshape(64, 1)
    w["bv2c"] = np.full((1, 1), float(np.asarray(inp["b_v2"]).ravel()[0]), np.float32)
    return w


def _prep_core(c, start, end, ef, nfi, nf_bf, cd_all, cdn_all, invcnt_all, b_n2):
    bfdt = mybir.dt.np(bf16)
    lo, hi = c * NPC, (c + 1) * NPC
    sel = (start >= lo) & (start < hi)
    eo = np.nonzero(sel)[0]
    s_loc = (start[eo] - lo).astype(np.int64)
    blk = s_loc >> 7
    order = np.argsort(blk, kind="stable")
    eo = eo[order]; s_loc = s_loc[order]; blk = blk[order]
    counts = np.bincount(blk, minlength=NBLK)
    if counts.max() > BLKE:
        raise RuntimeError(f"block overflow: {counts.max()} > {BLKE}")
    starts_ = np.zeros(NBLK, np.int64)
    starts_[1:] = np.cumsum(counts)[:-1]
    within = np.arange(len(eo)) - starts_[blk]
    slots = blk * BLKE + within

    sg = np.zeros(ES, np.int64)          # global start per slot (pad -> node lo)
    sg[:] = lo
    eg = np.zeros(ES, np.int64)          # global end per slot (pad -> node 0)
    lid = np.full(ES, -1.0, np.float32)
    cds = np.zeros((ES, 3), np.float32)
    cdns = np.zeros(ES, np.float32)
    efs = np.zeros((ES, EF), np.float32)
    sg[slots] = start[eo]
    eg[slots] = end[eo]
    lid[slots] = (s_loc & 127).astype(np.float32)
    cds[slots] = cd_all[eo] * invcnt_all[start[eo]][:, None]
    cdns[slots] = cdn_all[eo]
    efs[slots] = ef[eo]

    d = {}
    # [NBLK, 128, BLKE] bf16: rows 0:64 = nf[start].T, 64:128 = nf[end].T
    nfse = np.empty((ES, 128), bfdt)
    nfse[:, 0:64] = nf_bf[sg]
    nfse[:, 64:128] = nf_bf[eg]
    d["nfse"] = np.ascontiguousarray(
        nfse.reshape(NBLK, BLKE, 128).transpose(0, 2, 1))
    # scatter-orientation lid stream [NBLK, 128, NCH] (edge-in-chunk, chunk)
    lidc = lid.reshape(NBLK, NCH, 128)
    d["lidc"] = np.ascontiguousarray(lidc.transpose(0, 2, 1)).astype(bfdt)
    d["cdem"] = cds.reshape(NBLK, NCH, 128, 3).transpose(0, 2, 1, 3).copy()  # [NBLK,128,24,3]
    efr = efs.reshape(NBLK, 2, NSUP, 512, EF)
    cdnr = cdns.reshape(NBLK, 2, NSUP, 512)
    eA = np.concatenate([efr[:, 0].transpose(0, 1, 3, 2),
                         cdnr[:, 0][:, :, None, :]], axis=2)             # [NBLK,3,17,512]
    eB = np.concatenate([efr[:, 1].transpose(0, 1, 3, 2),
                         cdnr[:, 1][:, :, None, :]], axis=2)
    ecat = np.concatenate([eA, eB], axis=2)                              # [NBLK,3,34,512]
    d["efcdn"] = np.ascontiguousarray(
        ecat.transpose(0, 2, 1, 3).reshape(NBLK, 34, NSUP * 512)).astype(bfdt)

    nm = np.zeros((NPAD, 70), np.float32)
    nm[0:NPC] = nfi[lo:hi]
    nm[:, 6:70] += b_n2[None, :]
    d["node_nm"] = nm.reshape(NBLK, 128, 70).transpose(1, 0, 2).reshape(128, NBLK * 70).copy()
    nl = np.zeros((64, NPAD), np.float32)
    nl[:, 0:NPC] = nfi[lo:hi, 6:70].T
    d["nfT_local"] = nl
    return d


def _build_program():
    STAGE = int(os.environ.get("EGNN_STAGE", "5"))
    nc = bacc.Bacc("TRN2", target_bir_lowering=False, debug=False,
                   enable_asserts=False, num_devices=NCORES)

    def din(name, shape, dt=f32):
        return nc.dram_tensor(name, list(shape), dt, kind="ExternalInput").ap()

    nfse_d = din("nfse", [NBLK, 128, BLKE], bf16)
    lidc_d = din("lidc", [NBLK, 128, NCH], bf16)
    cdem_d = din("cdem", [NBLK, 128, NCH, 3])
    efcdn_d = din("efcdn", [NBLK, 34, NSUP * 512], bf16)
    node_nm_d = din("node_nm", [128, NBLK * 70])
    nfT_loc_d = din("nfT_local", [64, NPAD])
    wnames = ["wse", "wefcdn", "wde2", "wc1d", "rawcA", "rawcB",
              "wn1a", "wn1b", "wn2", "wv1", "wv2", "eye64", "arange128",
              "be1s", "be2s", "bc1s", "bih", "bn1c", "bv1c", "bv2c"]
    wshapes = {"wse": [128, 64], "wefcdn": [34, 128],
               "wde2": [128, 128], "wc1d": [128, 64],
               "rawcA": [128, 66], "rawcB": [128, 66],
               "wn1a": [64, 64], "wn1b": [64, 64], "wn2": [64, 64],
               "wv1": [64, 64], "wv2": [64, 1], "eye64": [64, 64],
               "arange128": [128, 128],
               "be1s": [128, 1], "be2s": [128, 1], "bc1s": [128, 1],
               "bih": [128, 1], "bn1c": [64, 1], "bv1c": [64, 1],
               "bv2c": [1, 1]}
    wd = {n: din(n, wshapes[n]) for n in wnames}
    out_d = nc.dram_tensor("out", [NPAD, 70], f32, kind="ExternalOutput").ap()

    # weights that are matmul operands in the bf16 pipeline
    BF_W = ("wse", "wefcdn", "wde2", "wc1d", "rawcA", "rawcB", "arange128")

    with tile.TileContext(nc) as tc, contextlib.ExitStack() as ctx:
        wpool = ctx.enter_context(tc.tile_pool(name="w", bufs=1))
        wt = {}
        for n in wnames:
            dt = bf16 if n in BF_W else f32
            t = wpool.tile(wshapes[n], dt, name=f"wt_{n}")
            if dt == f32:
                nc.sync.dma_start(t[:], wd[n][:])
            else:
                tf = wpool.tile(wshapes[n], f32, name=f"wtf_{n}")
                nc.sync.dma_start(tf[:], wd[n][:])
                nc.vector.tensor_copy(t[:], tf[:])
            wt[n] = t
        node_nm = wpool.tile([128, NBLK * 70], f32, name="node_nm")
        nc.sync.dma_start(node_nm[:], node_nm_d[:])
        nfT_loc = wpool.tile([64, NPAD], f32, name="nfT_loc")
        nc.sync.dma_start(nfT_loc[:], nfT_loc_d[:])
        vscale = wpool.tile([128, NBLK], f32, name="vscale")
        aggm = wpool.tile([64, NPAD], f32, name="aggm")
        aggc = wpool.tile([3, NPAD], f32, name="aggc")

        # ---------- Phase B: velocity MLP -> vscale [128, NBLK] ----------
        with tc.tile_pool(name="pb", bufs=2) as pb, \
             tc.tile_pool(name="pbp", bufs=2, space="PSUM") as pbp:
            tiles = [(j * 512, 512) for j in range(NPAD // 512)]
            if NPAD % 512:
                tiles.append((NPAD // 512 * 512, NPAD % 512))
            for (o, L) in tiles:
                vps = pbp.tile([64, L], f32, name=f"vps{o}", tag="vps")
                nc.tensor.matmul(vps[:], wt["wv1"][:], nfT_loc[:, o:o + L])
                vh = pb.tile([64, L], f32, name=f"vh{o}", tag="vh")
                nc.scalar.activation(vh[:], vps[:], AF_SILU, bias=wt["bv1c"][:])
                sps = pbp.tile([1, L], f32, name=f"sps{o}", tag="sps")
                nc.tensor.matmul(sps[:], wt["wv2"][:], vh[:])
                vsc = pb.tile([1, L], f32, name=f"vsc{o}", tag="vsc")
                nc.scalar.activation(vsc[:], sps[:], AF.Identity, bias=wt["bv2c"][:])
                for k in range(L // 128):
                    tp = pbp.tile([128, 1], f32, name=f"tp{o}_{k}", tag="tp")
                    nc.tensor.transpose(tp[:], vsc[:, k * 128:(k + 1) * 128],
                                        wt["eye64"][0:1, 0:1])
                    nc.vector.tensor_copy(vscale[:, o // 128 + k:o // 128 + k + 1], tp[:])

        # ---------- Edge sweep (phase C fused per block) ----------
        if STAGE >= 2:
            _edge_sweep(nc, tc, STAGE, wt, nfse_d, lidc_d,
                        cdem_d, efcdn_d, nfT_loc, aggm, aggc,
                        node_nm, vscale, out_d)

    nc.compile()
    return nc


def _edge_sweep(nc, tc, STAGE, wt, nfse_d, lidc_d,
                cdem_d, efcdn_d, nfT_loc, aggm, aggc,
                node_nm, vscale, out_d):
    with tc.tile_pool(name="pg", bufs=3) as pg, \
         tc.tile_pool(name="pe", bufs=3) as pe, \
         tc.tile_pool(name="pch", bufs=6) as pch, \
         tc.tile_pool(name="poh", bufs=2) as poh, \
         tc.tile_pool(name="px1", bufs=2, space="PSUM") as px1, \
         tc.tile_pool(name="pmc", bufs=3, space="PSUM") as pmc, \
         tc.tile_pool(name="pst", bufs=2, space="PSUM") as pst, \
         tc.tile_pool(name="pagg", bufs=1, space="PSUM") as pagg:

        def phase_c(b):
            cols = slice(b * 128, (b + 1) * 128)
            n1 = pagg.tile([64, 128], f32, name=f"n1{b}", tag="aggT")
            nc.tensor.matmul(n1[:], wt["wn1a"][:], nfT_loc[:, cols],
                             start=True, stop=False)
            nc.tensor.matmul(n1[:], wt["wn1b"][:], aggm[:, cols],
                             start=False, stop=True)
            hn = pch.tile([64, 128], f32, name=f"hn{b}", tag="hn")
            nc.scalar.activation(hn[:], n1[:], AF_SILU, bias=wt["bn1c"][:])
            n2 = pagg.tile([64, 128], f32, name=f"n2{b}", tag="aggT")
            nc.tensor.matmul(n2[:], wt["wn2"][:], hn[:])
            hn2 = pch.tile([64, 128], f32, name=f"hn2{b}", tag="hn2")
            nc.vector.tensor_copy(hn2[:], n2[:])
            nmb = node_nm[:, b * 70:(b + 1) * 70]
            ndel = pagg.tile([128, 64], f32, name=f"ndel{b}", tag="aggT")
            nc.tensor.transpose(ndel[:], hn2[:], wt["eye64"][:])
            ot67 = pch.tile([128, 64], f32, name=f"ot67{b}", tag="ot67")
            nc.vector.tensor_tensor(ot67[:], nmb[:, 6:70], ndel[:], OP.add)
            ctp = pagg.tile([128, 3], f32, name=f"ctp{b}", tag="aggT")
            nc.tensor.transpose(ctp[:], aggc[:, cols], wt["eye64"][0:3, 0:3])
            otc = pch.tile([128, 6], f32, name=f"otc{b}", tag="otc")
            t2 = pch.tile([128, 3], f32, name=f"t2{b}", tag="t2")
            nc.vector.tensor_tensor(
                t2[:], nmb[:, 3:6],
                vscale[:, b:b + 1].broadcast_to([128, 3]), OP.mult)
            nc.vector.tensor_copy(otc[:, 3:6], nmb[:, 3:6])
            t3 = pch.tile([128, 3], f32, name=f"t3{b}", tag="t3")
            nc.vector.tensor_tensor(t3[:], ctp[:], t2[:], OP.add)
            nc.vector.tensor_tensor(otc[:, 0:3], t3[:], nmb[:, 0:3], OP.add)
            nc.sync.dma_start(out_d[b * 128:(b + 1) * 128, 0:6], otc[:])
            nc.sync.dma_start(out_d[b * 128:(b + 1) * 128, 6:70], ot67[:])

        for b in range(NBLK):
            nfse = pg.tile([128, BLKE], bf16, name=f"nfse{b}", tag="nfse")
            nc.sync.dma_start(nfse[:], nfse_d[b])
            lidt = pg.tile([128, NCH], bf16, name=f"lidt{b}", tag="lidt")
            nc.sync.dma_start(lidt[:], lidc_d[b])
            cdt = pg.tile([128, NCH, 3], f32, name=f"cdt{b}", tag="cdt")
            nc.sync.dma_start(cdt[:], cdem_d[b])
            eftb = pg.tile([34, NSUP * 512], bf16, name=f"eftb{b}", tag="eftb")
            nc.sync.dma_start(eftb[:], efcdn_d[b])

            # scatter one-hot [128 edge-in-chunk, 24 chunk, 128 node] on-chip
            ohts = poh.tile([128, NCH, 128], bf16, name=f"ohts{b}", tag="ohts")
            nc.vector.tensor_tensor(
                ohts[:],
                lidt[:].unsqueeze(2).broadcast_to([128, NCH, 128]),
                wt["arange128"][:].unsqueeze(1).broadcast_to([128, NCH, 128]),
                OP.is_equal)

            if STAGE == 2:
                nc.any.tensor_copy(aggm[:, b * 128:(b + 1) * 128],
                                   ohts[0:64, 0, :])
                continue

            # ---- stage 1: x1 accumulation + first silu (per supertile) ----
            h1s = []
            for s in range(NSUP):
                sl = slice(s * 512, (s + 1) * 512)
                slh = slice(NSUP * 512 + s * 512, NSUP * 512 + (s + 1) * 512)
                x1 = px1.tile([128, 512], f32, name=f"x1{b}_{s}", tag="x1")
                nc.tensor.matmul(x1[0:64, :], wt["wse"][:], nfse[:, sl],
                                 start=True, stop=False, skip_group_check=True)
                nc.tensor.matmul(x1[64:128, :], wt["wse"][:], nfse[:, slh],
                                 start=True, stop=False,
                                 tile_position=(0, 64), skip_group_check=True)
                nc.tensor.matmul(x1[:], wt["wefcdn"][:],
                                 eftb[:, s * 512:(s + 1) * 512],
                                 start=False, stop=True, skip_group_check=True)
                h1 = pe.tile([128, 512], bf16, name=f"h1{b}_{s}", tag="h1")
                nc.scalar.activation(h1[:], x1[:], AF_SILU, bias=wt["be1s"][:])
                h1s.append(h1)

            # previous block's node update fills engine bubbles here
            if STAGE >= 5 and b > 0:
                phase_c(b - 1)

            # ---- stage 2+3: message silu into mc halves; coord-hidden via
            # two concurrent diagonal-quadrant K=64 matmuls; then ch silu.
            # mcA = [msgA ; chA], mcB = [chB ; msgB]
            mcs = []
            for s in range(NSUP):
                mp = pmc.tile([128, 512], f32, name=f"mp{b}_{s}", tag="mmid")
                nc.tensor.matmul(mp[:], wt["wde2"][:], h1s[s][:])
                mcA = pe.tile([128, 512], bf16, name=f"mcA{b}_{s}", tag="mcA")
                mcB = pe.tile([128, 512], bf16, name=f"mcB{b}_{s}", tag="mcB")
                nc.scalar.activation(mcA[0:64, :], mp[0:64, :], AF_SILU,
                                     bias=wt["be2s"][0:64, :])
                nc.scalar.activation(mcB[64:128, :], mp[64:128, :], AF_SILU,
                                     bias=wt["be2s"][64:128, :])
                cpx = pmc.tile([128, 512], f32, name=f"cp{b}_{s}", tag="mmid")
                nc.tensor.matmul(cpx[64:128, :], wt["wc1d"][0:64, :], mcA[0:64, :],
                                 start=True, stop=True,
                                 tile_position=(0, 64), skip_group_check=True)
                nc.tensor.matmul(cpx[0:64, :], wt["wc1d"][64:128, :], mcB[64:128, :],
                                 start=True, stop=True,
                                 tile_position=(64, 0), skip_group_check=True)
                nc.scalar.activation(mcA[64:128, :], cpx[64:128, :], AF_SILU,
                                     bias=wt["bc1s"][64:128, :])
                nc.scalar.activation(mcB[0:64, :], cpx[0:64, :], AF_SILU,
                                     bias=wt["bc1s"][0:64, :])
                mcs.append((mcA, mcB))

            if STAGE == 3:
                nc.any.tensor_copy(aggm[:, b * 128:(b + 1) * 128],
                                   mcs[0][0][0:64, 0:128])
                continue

            # ---- stage 4: edge-major flip (msg|gate|coord) + gate ----
            # rcg [128 edge, 24 chunk, 67]: cols 0:64 msg*gate, 64:67 coord
            rcg = pch.tile([128, NCH, 67], bf16, name=f"rcg{b}", tag="rcg")
            for g4 in range(NCH // GRP):
                st = pst.tile([128, GRP, 66], f32, name=f"st{b}_{g4}", tag="st")
                for cg in range(GRP):
                    j = g4 * GRP + cg                 # chunk 0..23
                    if j < NCH // 2:
                        mc = mcs[j // 4][0]           # A half
                        rawc = wt["rawcA"]
                        cc = slice((j % 4) * 128, (j % 4 + 1) * 128)
                    else:
                        mc = mcs[(j - NCH // 2) // 4][1]
                        rawc = wt["rawcB"]
                        cc = slice(((j - NCH // 2) % 4) * 128,
                                   ((j - NCH // 2) % 4 + 1) * 128)
                    nc.tensor.matmul(st[:, cg, :], mc[:, cc], rawc[:],
                                     start=True, stop=True)
                jb = slice(g4 * GRP, (g4 + 1) * GRP)
                tnh = pch.tile([128, GRP], f32, name=f"tnh{b}_{g4}", tag="tnh")
                nc.scalar.activation(tnh[:], st[:, :, 64:65].squeeze(2),
                                     AF.Tanh, bias=wt["bih"][:], scale=0.5)
                gate = pch.tile([128, GRP], f32, name=f"gt{b}_{g4}", tag="gate")
                nc.vector.tensor_scalar(out=gate[:], in0=tnh[:], scalar1=1.0,
                                        scalar2=0.5, op0=OP.add, op1=OP.mult)
                nc.vector.tensor_tensor(
                    rcg[:, jb, 0:64], st[:, :, 0:64],
                    gate[:].unsqueeze(2).broadcast_to([128, GRP, 64]), OP.mult)
                nc.vector.tensor_tensor(
                    rcg[:, jb, 64:67], cdt[:, jb, :],
                    st[:, :, 65:66].broadcast_to([128, GRP, 3]), OP.mult)

            # ---- stage 5: segment-sum scatter into aggT [67, 128 nodes] ----
            # rows 0:64 msg-sum, 64:67 coord-sum
            aggT = pagg.tile([67, 128], f32, name=f"aggT{b}", tag="aggT")
            for j in range(NCH):
                nc.tensor.matmul(aggT[:, :], rcg[:, j, :],
                                 ohts[:, j, :],
                                 start=(j == 0), stop=(j == NCH - 1))
            nc.scalar.activation(aggm[:, b * 128:(b + 1) * 128], aggT[0:64, :], AF.Identity)
            nc.scalar.activation(aggc[:, b * 128:(b + 1) * 128], aggT[64:67, :], AF.Identity)

        if STAGE >= 5:
            phase_c(NBLK - 1)


def kernel(**inputs):
    ei = np.asarray(inputs["edge_indices"])
    start = ei[0].astype(np.int64)
    end = ei[1].astype(np.int64)
    ef = _f(inputs["edge_features"])
    nfi = _f(inputs["node_features_input"])
    coords = nfi[:, 0:3]
    cd_all = coords[start] - coords[end]
    cdn_all = np.sqrt((cd_all ** 2).sum(1)).astype(np.float32)
    deg = np.bincount(start, minlength=N).astype(np.float32)
    invcnt_all = (1.0 / np.maximum(deg, 1.0)).astype(np.float32)
    nf_bf = nfi[:, 6:70].astype(mybir.dt.np(bf16))

    w = _prep_weights(inputs)

    in_maps = []
    for c in range(NCORES):
        d = _prep_core(c, start, end, ef, nfi, nf_bf, cd_all, cdn_all,
                       invcnt_all, _f(inputs["b_n2"]))
        d.update(w)
        in_maps.append(d)

    if "nc" not in _cache:
        _cache["nc"] = _build_program()
    nc = _cache["nc"]
    _cache["in_maps"] = in_maps
    res = run_bass_kernel_spmd(nc, in_maps, list(range(NCORES)))
    out = np.empty((N, 70), np.float32)
    for c in range(NCORES):
        out[c * NPC:(c + 1) * NPC] = res.results[c]["out"][0:NPC]
    return out


# revision 13
# speedup vs baseline: 2.3248x; 1.1064x over previous
"""EquivariantGraphConvolution (EGNN layer) on 8 Trainium2 NeuronCores.

Strategy (v3.2)
---------------
Nodes are range-partitioned across the 8 cores (6250 nodes each); every edge is
owned by the core that owns its *start* node, so the per-start segment sums are
core-local and no collective is needed.  Per core, edges are bucketed by
128-node start block and padded to a static layout of 49 blocks x NCH chunks x
128 edges, NCH chosen from the data (ceil(max block edges / 128), even).

Key design points (see earlier versions for the v2 gather design):
  * No dma_gather anywhere: the host materializes [nf[start] ; nf[end]] per
    edge slot as one [128, NCH*128] bf16 stream per block (128B/edge).
    Stage 1 is 2 col-tiled matmuls with the [W1a; W1b] stationary plus one
    for the ef/coord-norm terms.
  * The scatter one-hot is built on-chip with one DVE is_equal per block.
  * Segment-sum: rcg [128 edges, 67] bf16 is the matmul stationary, one-hot
    moving, accumulating aggT [67 feats, 128 nodes] over NCH chunks.
  * Activation-table discipline: only Silu/Tanh/Identity (one table set).
    Gate = 0.5*(1+tanh(0.5 logit + 0.5 b_i)).
  * 1/deg is folded into the coord stream, b_n2 into a DVE add; the nf
    output is written feature-major so no transposes are needed in the
    node-update phase; velocity scales go through a DRAM round-trip to
    avoid 49 PE transposes.
"""
import sys
sys.path.insert(0, "/opt/trn_rl_repo")
import contextlib
import os
import numpy as np

import concourse.bass as bass
import concourse.bacc as bacc
import concourse.mybir as mybir
import concourse.tile as tile
from concourse.bass_utils import run_bass_kernel_spmd

f32 = mybir.dt.float32
bf16 = mybir.dt.bfloat16
AF = mybir.ActivationFunctionType
OP = mybir.AluOpType

# ---- problem constants (hardcoded per contract) ----
N = 50000
E = 1_000_000
H = 64
EF = 16
NCORES = 8
NPC = N // NCORES          # 6250 nodes per core
NBLK = 49                  # 128-node blocks per core (49*128 = 6272 >= 6250)
NPAD = NBLK * 128          # 6272
GRP = 6                    # max chunks per flip/gate group

# CoreSim does not implement Silu; substitute Tanh for structural sim checks.
SIM_ACT = os.environ.get("EGNN_SIMACT", "0") == "1"
AF_SILU = AF.Tanh if SIM_ACT else AF.Silu

_cache = {}


def _sup_widths(half):
    """Supertile widths covering `half` columns (each <= 512, mult of 128)."""
    w = [512] * (half // 512)
    if half % 512:
        w.append(half % 512)
    return w


def _f(x):
    return np.ascontiguousarray(x, np.float32)


def _prep_weights(inp):
    """Small weight/constant tensors, identical on all cores."""
    W_e1 = _f(inp["W_e1"])           # [145, 64]
    w = {}
    w["wse"] = W_e1[0:128].copy()                    # [128, 64] start|end rows
    wef = np.zeros((34, 128), np.float32)
    wef[0:16, 0:64] = W_e1[129:145]
    wef[16:17, 0:64] = W_e1[128:129]
    wef[17:33, 64:128] = W_e1[129:145]
    wef[33:34, 64:128] = W_e1[128:129]
    w["wefcdn"] = wef
    W_e2 = _f(inp["W_e2"]); W_c1 = _f(inp["W_c1"])
    bd = np.zeros((128, 128), np.float32)
    bd[0:64, 0:64] = W_e2; bd[64:128, 64:128] = W_e2
    w["wde2"] = bd
    w["wc1d"] = np.concatenate([W_c1, W_c1], axis=0)   # [128, 64], both halves
    W_i = _f(inp["W_i"]); W_c2 = _f(inp["W_c2"])
    # mcA = [msgA ; chA]  -> out cols: 0:64 msg, 64 gate logit, 65 coord weight
    ra = np.zeros((128, 66), np.float32)
    ra[0:64, 0:64] = np.eye(64); ra[0:64, 64:65] = W_i; ra[64:128, 65:66] = W_c2
    w["rawcA"] = ra
    # mcB = [chB ; msgB]
    rb = np.zeros((128, 66), np.float32)
    rb[64:128, 0:64] = np.eye(64); rb[64:128, 64:65] = W_i; rb[0:64, 65:66] = W_c2
    w["rawcB"] = rb
    w["wn1a"] = _f(inp["W_n1"])[0:64].copy()          # [64, 64] nf rows
    w["wn1b"] = _f(inp["W_n1"])[64:128].copy()        # [64, 64] agg rows
    w["wn2"] = _f(inp["W_n2"])
    w["wv1"] = _f(inp["W_v1"])
    w["wv2"] = _f(inp["W_v2"])
    w["eye3"] = np.eye(3, dtype=np.float32)
    w["arange128"] = np.tile(np.arange(128, dtype=np.float32), (128, 1))

    b_e1 = _f(inp["b_e1"]); b_e2 = _f(inp["b_e2"]); b_c1 = _f(inp["b_c1"])
    w["be1s"] = np.concatenate([b_e1, b_e1]).reshape(128, 1)
    w["be2s"] = np.concatenate([b_e2, b_e2]).reshape(128, 1)
    w["bc1s"] = np.concatenate([b_c1, b_c1]).reshape(128, 1)
    w["bih"] = np.full((128, 1), 0.5 * float(np.asarray(inp["b_i"]).ravel()[0]), np.float32)
    w["bn1c"] = _f(inp["b_n1"]).reshape(64, 1)
    w["bn2c"] = _f(inp["b_n2"]).reshape(64, 1)
    w["bv1c"] = _f(inp["b_v1"]).reshape(64, 1)
    w["bv2c"] = np.full((1, 1), float(np.asarray(inp["b_v2"]).ravel()[0]), np.float32)
    return w


def _prep_core(c, NCH, start, end, ef, nfi, nf_bf, cd_all, cdn_all, invcnt_all):
    bfdt = mybir.dt.np(bf16)
    BLKE = NCH * 128
    ES = NBLK * BLKE
    HALF = BLKE // 2
    lo, hi = c * NPC, (c + 1) * NPC
    sel = (start >= lo) & (start < hi)
    eo = np.nonzero(sel)[0]
    s_loc = (start[eo] - lo).astype(np.int64)
    blk = s_loc >> 7
    order = np.argsort(blk, kind="stable")
    eo = eo[order]; s_loc = s_loc[order]; blk = blk[order]
    counts = np.bincount(blk, minlength=NBLK)
    if counts.max() > BLKE:
        raise RuntimeError(f"block overflow: {counts.max()} > {BLKE}")
    starts_ = np.zeros(NBLK, np.int64)
    starts_[1:] = np.cumsum(counts)[:-1]
    within = np.arange(len(eo)) - starts_[blk]
    slots = blk * BLKE + within

    sg = np.full(ES, lo, np.int64)       # global start per slot (pad -> node lo)
    eg = np.zeros(ES, np.int64)          # global end per slot (pad -> node 0)
    lid = np.full(ES, -1.0, np.float32)
    cds = np.zeros((ES, 3), np.float32)
    cdns = np.zeros(ES, np.float32)
    efs = np.zeros((ES, EF), np.float32)
    sg[slots] = start[eo]
    eg[slots] = end[eo]
    lid[slots] = (s_loc & 127).astype(np.float32)
    cds[slots] = cd_all[eo] * invcnt_all[start[eo]][:, None]
    cdns[slots] = cdn_all[eo]
    efs[slots] = ef[eo]

    d = {}
    # [NBLK, 128, BLKE] bf16: rows 0:64 = nf[start].T, 64:128 = nf[end].T
    nfse = np.empty((ES, 128), bfdt)
    nfse[:, 0:64] = nf_bf[sg]
    nfse[:, 64:128] = nf_bf[eg]
    d["nfse"] = np.ascontiguousarray(
        nfse.reshape(NBLK, BLKE, 128).transpose(0, 2, 1))
    # scatter-orientation lid stream [NBLK, 128, NCH] (edge-in-chunk, chunk)
    lidc = lid.reshape(NBLK, NCH, 128)
    d["lidc"] = np.ascontiguousarray(lidc.transpose(0, 2, 1)).astype(bfdt)
    d["cdem"] = cds.reshape(NBLK, NCH, 128, 3).transpose(0, 2, 1, 3).copy()
    # ef|cdn feature-major: rows 0:17 = A-half edge, 17:34 = B-half edge
    efcdn = np.empty((NBLK, 2, HALF, EF + 1), np.float32)
    efcdn[:, :, :, 0:EF] = efs.reshape(NBLK, 2, HALF, EF)
    efcdn[:, :, :, EF] = cdns.reshape(NBLK, 2, HALF)
    d["efcdn"] = np.ascontiguousarray(
        efcdn.transpose(0, 1, 3, 2).reshape(NBLK, 34, HALF)).astype(bfdt)

    nmc = np.zeros((NPAD, 6), np.float32)
    nmc[0:NPC] = nfi[lo:hi, 0:6]
    d["nodec"] = nmc.reshape(NBLK, 128, 6).transpose(1, 0, 2).reshape(128, NBLK * 6).copy()
    nl = np.zeros((64, NPAD), np.float32)
    nl[:, 0:NPC] = nfi[lo:hi, 6:70].T
    d["nfT_local"] = nl
    return d


def _build_program(NCH):
    STAGE = int(os.environ.get("EGNN_STAGE", "5"))
    BLKE = NCH * 128
    HALF = BLKE // 2
    SUPW = _sup_widths(HALF)        # supertile widths
    SUPO = [sum(SUPW[:i]) for i in range(len(SUPW))]
    # flip/gate groups: chunks [o, o+n) per group
    GRPS = []
    o = 0
    while o < NCH:
        n = min(GRP, NCH - o)
        GRPS.append((o, n))
        o += n

    nc = bacc.Bacc("TRN2", target_bir_lowering=False, debug=False,
                   enable_asserts=False, num_devices=NCORES)

    def din(name, shape, dt=f32):
        return nc.dram_tensor(name, list(shape), dt, kind="ExternalInput").ap()

    nfse_d = din("nfse", [NBLK, 128, BLKE], bf16)
    lidc_d = din("lidc", [NBLK, 128, NCH], bf16)
    cdem_d = din("cdem", [NBLK, 128, NCH, 3])
    efcdn_d = din("efcdn", [NBLK, 34, HALF], bf16)
    nodec_d = din("nodec", [128, NBLK * 6])
    nfT_loc_d = din("nfT_local", [64, NPAD])
    wnames = ["wse", "wefcdn", "wde2", "wc1d", "rawcA", "rawcB",
              "wn1a", "wn1b", "wn2", "wv1", "wv2", "eye3", "arange128",
              "be1s", "be2s", "bc1s", "bih", "bn1c", "bn2c", "bv1c", "bv2c"]
    wshapes = {"wse": [128, 64], "wefcdn": [34, 128],
               "wde2": [128, 128], "wc1d": [128, 64],
               "rawcA": [128, 66], "rawcB": [128, 66],
               "wn1a": [64, 64], "wn1b": [64, 64], "wn2": [64, 64],
               "wv1": [64, 64], "wv2": [64, 1], "eye3": [3, 3],
               "arange128": [128, 128],
               "be1s": [128, 1], "be2s": [128, 1], "bc1s": [128, 1],
               "bih": [128, 1], "bn1c": [64, 1], "bn2c": [64, 1],
               "bv1c": [64, 1], "bv2c": [1, 1]}
    wd = {n: din(n, wshapes[n]) for n in wnames}
    outc_d = nc.dram_tensor("outc", [NPAD, 6], f32, kind="ExternalOutput").ap()
    outT_d = nc.dram_tensor("outT", [64, NPAD], f32, kind="ExternalOutput").ap()
    vs_dram = nc.dram_tensor("vs_dram", [NPAD], f32).ap()

    # weights that are matmul operands in the bf16 pipeline
    BF_W = ("wse", "wefcdn", "wde2", "wc1d", "rawcA", "rawcB", "arange128")

    with tile.TileContext(nc) as tc, contextlib.ExitStack() as ctx:
        wpool = ctx.enter_context(tc.tile_pool(name="w", bufs=1))
        wt = {}
        for n in wnames:
            dt = bf16 if n in BF_W else f32
            t = wpool.tile(wshapes[n], dt, name=f"wt_{n}")
            if dt == f32:
                nc.sync.dma_start(t[:], wd[n][:])
            else:
                tf = wpool.tile(wshapes[n], f32, name=f"wtf_{n}")
                nc.sync.dma_start(tf[:], wd[n][:])
                nc.vector.tensor_copy(t[:], tf[:])
            wt[n] = t
        nodec = wpool.tile([128, NBLK * 6], f32, name="nodec")
        nc.sync.dma_start(nodec[:], nodec_d[:])
        nfT_loc = wpool.tile([64, NPAD], f32, name="nfT_loc")
        nc.sync.dma_start(nfT_loc[:], nfT_loc_d[:])
        vscale = wpool.tile([128, NBLK], f32, name="vscale")
        aggm = wpool.tile([64, NPAD], f32, name="aggm")
        aggc = wpool.tile([3, NPAD], f32, name="aggc")

        # ---------- Phase B: velocity MLP -> vscale [128, NBLK] ----------
        with tc.tile_pool(name="pb", bufs=2) as pb, \
             tc.tile_pool(name="pbp", bufs=2, space="PSUM") as pbp:
            tiles = [(j * 512, 512) for j in range(NPAD // 512)]
            if NPAD % 512:
                tiles.append((NPAD // 512 * 512, NPAD % 512))
            for (o, L) in tiles:
                vps = pbp.tile([64, L], f32, name=f"vps{o}", tag="vps")
                nc.tensor.matmul(vps[:], wt["wv1"][:], nfT_loc[:, o:o + L])
                vh = pb.tile([64, L], f32, name=f"vh{o}", tag="vh")
                nc.scalar.activation(vh[:], vps[:], AF_SILU, bias=wt["bv1c"][:])
                sps = pbp.tile([1, L], f32, name=f"sps{o}", tag="sps")
                nc.tensor.matmul(sps[:], wt["wv2"][:], vh[:])
                vsc = pb.tile([1, L], f32, name=f"vsc{o}", tag="vsc")
                nc.scalar.activation(vsc[:], sps[:], AF.Identity, bias=wt["bv2c"][:])
                nc.sync.dma_start(vs_dram[o:o + L].unsqueeze(0), vsc[:])
            # read back node-block-major: vscale[p, b] = vs_dram[b*128 + p]
            nc.sync.dma_start(vscale[:],
                              vs_dram[:].rearrange("(b p) -> p b", p=128))

        # ---------- Edge sweep (node update fused per block) ----------
        if STAGE >= 2:
            _edge_sweep(nc, tc, STAGE, NCH, SUPW, SUPO, GRPS, wt,
                        nfse_d, lidc_d, cdem_d, efcdn_d, nfT_loc,
                        aggm, aggc, nodec, vscale, outc_d, outT_d)

    nc.compile()
    return nc


def _edge_sweep(nc, tc, STAGE, NCH, SUPW, SUPO, GRPS, wt,
                nfse_d, lidc_d, cdem_d, efcdn_d, nfT_loc,
                aggm, aggc, nodec, vscale, outc_d, outT_d):
    BLKE = NCH * 128
    HALF = BLKE // 2
    NHC = NCH // 2
    NSUP = len(SUPW)

    def mc_of_chunk(mcs, j):
        """Map chunk j to (mc tile, rawc, 128-col slice within supertile)."""
        half, jj = (0, j) if j < NHC else (1, j - NHC)
        col = jj * 128
        for s in range(NSUP):
            if col < SUPO[s] + SUPW[s]:
                off = col - SUPO[s]
                rawc = wt["rawcA"] if half == 0 else wt["rawcB"]
                return mcs[s][half], rawc, slice(off, off + 128)
        raise AssertionError

    with tc.tile_pool(name="pg", bufs=3) as pg, \
         tc.tile_pool(name="pe", bufs=3) as pe, \
         tc.tile_pool(name="pch", bufs=6) as pch, \
         tc.tile_pool(name="poh", bufs=2) as poh, \
         tc.tile_pool(name="px1", bufs=2, space="PSUM") as px1, \
         tc.tile_pool(name="pmc", bufs=2, space="PSUM") as pmc, \
         tc.tile_pool(name="pst", bufs=3, space="PSUM") as pst, \
         tc.tile_pool(name="pagg", bufs=1, space="PSUM") as pagg:

        def phase_c(b):
            cols = slice(b * 128, (b + 1) * 128)
            n1 = pagg.tile([64, 128], f32, name=f"n1{b}", tag="aggT")
            nc.tensor.matmul(n1[:], wt["wn1a"][:], nfT_loc[:, cols],
                             start=True, stop=False)
            nc.tensor.matmul(n1[:], wt["wn1b"][:], aggm[:, cols],
                             start=False, stop=True)
            hn = pch.tile([64, 128], f32, name=f"hn{b}", tag="hn")
            nc.scalar.activation(hn[:], n1[:], AF_SILU, bias=wt["bn1c"][:])
            n2 = pagg.tile([64, 128], f32, name=f"n2{b}", tag="aggT")
            nc.tensor.matmul(n2[:], wt["wn2"][:], hn[:])
            hn2 = pch.tile([64, 128], f32, name=f"hn2{b}", tag="hn2")
            nc.vector.tensor_tensor(
                hn2[:], n2[:], wt["bn2c"][:].broadcast_to([64, 128]), OP.add)
            ot67 = pch.tile([64, 128], f32, name=f"ot67{b}", tag="ot67")
            nc.vector.tensor_tensor(ot67[:], nfT_loc[:, cols], hn2[:], OP.add)
            nc.sync.dma_start(outT_d[:, cols], ot67[:])
            # coords/vels (node-major)
            nmb = nodec[:, b * 6:(b + 1) * 6]
            ctp = pagg.tile([128, 3], f32, name=f"ctp{b}", tag="aggT")
            nc.tensor.transpose(ctp[:], aggc[:, cols], wt["eye3"][:])
            otc = pch.tile([128, 6], f32, name=f"otc{b}", tag="otc")
            t2 = pch.tile([128, 3], f32, name=f"t2{b}", tag="t2")
            nc.vector.tensor_tensor(
                t2[:], nmb[:, 3:6],
                vscale[:, b:b + 1].broadcast_to([128, 3]), OP.mult)
            nc.vector.tensor_copy(otc[:, 3:6], nmb[:, 3:6])
            t3 = pch.tile([128, 3], f32, name=f"t3{b}", tag="t3")
            nc.vector.tensor_tensor(t3[:], ctp[:], t2[:], OP.add)
            nc.vector.tensor_tensor(otc[:, 0:3], t3[:], nmb[:, 0:3], OP.add)
            nc.sync.dma_start(outc_d[b * 128:(b + 1) * 128, :], otc[:])

        for b in range(NBLK):
            nfse = pg.tile([128, BLKE], bf16, name=f"nfse{b}", tag="nfse")
            nc.sync.dma_start(nfse[:], nfse_d[b])
            lidt = pg.tile([128, NCH], bf16, name=f"lidt{b}", tag="lidt")
            nc.sync.dma_start(lidt[:], lidc_d[b])
            cdt = pg.tile([128, NCH, 3], f32, name=f"cdt{b}", tag="cdt")
            nc.sync.dma_start(cdt[:], cdem_d[b])
            eftb = pg.tile([34, HALF], bf16, name=f"eftb{b}", tag="eftb")
            nc.sync.dma_start(eftb[:], efcdn_d[b])

            # scatter one-hot [128 edge-in-chunk, NCH chunk, 128 node] on-chip
            ohts = poh.tile([128, NCH, 128], bf16, name=f"ohts{b}", tag="ohts")
            nc.vector.tensor_tensor(
                ohts[:],
                lidt[:].unsqueeze(2).broadcast_to([128, NCH, 128]),
                wt["arange128"][:].unsqueeze(1).broadcast_to([128, NCH, 128]),
                OP.is_equal)

            if STAGE == 2:
                nc.any.tensor_copy(aggm[:, b * 128:(b + 1) * 128],
                                   ohts[0:64, 0, :])
                continue

            # ---- stage 1: x1 accumulation + first silu (per supertile) ----
            h1s = []
            for s in range(NSUP):
                w = SUPW[s]
                sl = slice(SUPO[s], SUPO[s] + w)
                slh = slice(HALF + SUPO[s], HALF + SUPO[s] + w)
                x1 = px1.tile([128, w], f32, name=f"x1{b}_{s}", tag="x1")
                nc.tensor.matmul(x1[0:64, :], wt["wse"][:], nfse[:, sl],
                                 start=True, stop=False, skip_group_check=True)
                nc.tensor.matmul(x1[64:128, :], wt["wse"][:], nfse[:, slh],
                                 start=True, stop=False,
                                 tile_position=(0, 64), skip_group_check=True)
                nc.tensor.matmul(x1[:], wt["wefcdn"][:], eftb[:, sl],
                                 start=False, stop=True, skip_group_check=True)
                h1 = pe.tile([128, w], bf16, name=f"h1{b}_{s}", tag="h1")
                nc.scalar.activation(h1[:], x1[:], AF_SILU, bias=wt["be1s"][:])
                h1s.append(h1)

            # previous block's node update fills engine bubbles here
            if STAGE >= 5 and b > 0:
                phase_c(b - 1)

            # ---- stage 2+3: message silu into mc halves; coord-hidden via
            # two concurrent diagonal-quadrant K=64 matmuls; then ch silu.
            # mcA = [msgA ; chA], mcB = [chB ; msgB]
            mcs = []
            for s in range(NSUP):
                w = SUPW[s]
                mp = pmc.tile([128, w], f32, name=f"mp{b}_{s}", tag="mmid")
                nc.tensor.matmul(mp[:], wt["wde2"][:], h1s[s][:])
                mcA = pe.tile([128, w], bf16, name=f"mcA{b}_{s}", tag="mcA")
                mcB = pe.tile([128, w], bf16, name=f"mcB{b}_{s}", tag="mcB")
                nc.scalar.activation(mcA[0:64, :], mp[0:64, :], AF_SILU,
                                     bias=wt["be2s"][0:64, :])
                nc.scalar.activation(mcB[64:128, :], mp[64:128, :], AF_SILU,
                                     bias=wt["be2s"][64:128, :])
                cpx = pmc.tile([128, w], f32, name=f"cp{b}_{s}", tag="mmid")
                nc.tensor.matmul(cpx[64:128, :], wt["wc1d"][0:64, :], mcA[0:64, :],
                                 start=True, stop=True,
                                 tile_position=(0, 64), skip_group_check=True)
                nc.tensor.matmul(cpx[0:64, :], wt["wc1d"][64:128, :], mcB[64:128, :],
                                 start=True, stop=True,
                                 tile_position=(64, 0), skip_group_check=True)
                nc.scalar.activation(mcA[64:128, :], cpx[64:128, :], AF_SILU,
                                     bias=wt["bc1s"][64:128, :])
                nc.scalar.activation(mcB[0:64, :], cpx[0:64, :], AF_SILU,
                                     bias=wt["bc1s"][0:64, :])
                mcs.append((mcA, mcB))

            if STAGE == 3:
                nc.any.tensor_copy(aggm[:, b * 128:(b + 1) * 128],
                                   mcs[0][0][0:64, 0:128])
                continue

            # ---- stage 4: edge-major flip (msg|gate|coord) + gate ----
            # rcg [128 edge, NCH chunk, 67]: cols 0:64 msg*gate, 64:67 coord
            rcg = pch.tile([128, NCH, 67], bf16, name=f"rcg{b}", tag="rcg")
            for (go, gn) in GRPS:
                st = pst.tile([128, gn, 66], f32, name=f"st{b}_{go}", tag="st")
                for cg in range(gn):
                    mc, rawc, cc = mc_of_chunk(mcs, go + cg)
                    nc.tensor.matmul(st[:, cg, :], mc[:, cc], rawc[:],
                                     start=True, stop=True)
                jb = slice(go, go + gn)
                tnh = pch.tile([128, gn], f32, name=f"tnh{b}_{go}", tag="tnh")
                nc.scalar.activation(tnh[:], st[:, :, 64:65].squeeze(2),
                                     AF.Tanh, bias=wt["bih"][:], scale=0.5)
                gate = pch.tile([128, gn], f32, name=f"gt{b}_{go}", tag="gate")
                nc.vector.tensor_scalar(out=gate[:], in0=tnh[:], scalar1=1.0,
                                        scalar2=0.5, op0=OP.add, op1=OP.mult)
                nc.vector.tensor_tensor(
                    rcg[:, jb, 0:64], st[:, :, 0:64],
                    gate[:].unsqueeze(2).broadcast_to([128, gn, 64]), OP.mult)
                nc.vector.tensor_tensor(
                    rcg[:, jb, 64:67], cdt[:, jb, :],
                    st[:, :, 65:66].broadcast_to([128, gn, 3]), OP.mult)

            # ---- stage 5: segment-sum scatter into aggT [67, 128 nodes] ----
            # rows 0:64 msg-sum, 64:67 coord-sum
            aggT = pagg.tile([67, 128], f32, name=f"aggT{b}", tag="aggT")
            for j in range(NCH):
                nc.tensor.matmul(aggT[:, :], rcg[:, j, :],
                                 ohts[:, j, :],
                                 start=(j == 0), stop=(j == NCH - 1))
            nc.scalar.activation(aggm[:, b * 128:(b + 1) * 128], aggT[0:64, :],
                                 AF.Identity)
            nc.scalar.activation(aggc[:, b * 128:(b + 1) * 128], aggT[64:67, :],
                                 AF.Identity)

        if STAGE >= 5:
            phase_c(NBLK - 1)


def kernel(**inputs):
    ei = np.asarray(inputs["edge_indices"])
    start = ei[0].astype(np.int64)
    end = ei[1].astype(np.int64)
    ef = _f(inputs["edge_features"])
    nfi = _f(inputs["node_features_input"])
    coords = nfi[:, 0:3]
    cd_all = coords[start] - coords[end]
    cdn_all = np.sqrt((cd_all ** 2).sum(1)).astype(np.float32)
    deg = np.bincount(start, minlength=N).astype(np.float32)
    invcnt_all = (1.0 / np.maximum(deg, 1.0)).astype(np.float32)
    nf_bf = nfi[:, 6:70].astype(mybir.dt.np(bf16))

    # chunk count from data: ceil(max block load / 128), rounded up to even
    mx = 0
    for c in range(NCORES):
        lo, hi = c * NPC, (c + 1) * NPC
        s = start[(start >= lo) & (start < hi)] - lo
        cnt = np.bincount(s >> 7, minlength=NBLK)
        mx = max(mx, int(cnt.max()))
    NCH = -(-mx // 128)
    NCH += NCH % 2
    NCH = max(NCH, 8)

    w = _prep_weights(inputs)

    in_maps = []
    for c in range(NCORES):
        d = _prep_core(c, NCH, start, end, ef, nfi, nf_bf, cd_all, cdn_all,
                       invcnt_all)
        d.update(w)
        in_maps.append(d)

    if _cache.get("NCH") != NCH:
        _cache["NCH"] = NCH
        _cache["nc"] = _build_program(NCH)
    nc = _cache["nc"]
    _cache["in_maps"] = in_maps
    res = run_bass_kernel_spmd(nc, in_maps, list(range(NCORES)))
    out = np.empty((N, 70), np.float32)
    for c in range(NCORES):
        out[c * NPC:(c + 1) * NPC, 0:6] = res.results[c]["outc"][0:NPC]
        out[c * NPC:(c + 1) * NPC, 6:70] = res.results[c]["outT"][:, 0:NPC].T
    return out


# revision 15
# speedup vs baseline: 2.7589x; 1.1867x over previous
"""EquivariantGraphConvolution (EGNN layer) on 8 Trainium2 NeuronCores.

Strategy (v3.2)
---------------
Nodes are range-partitioned across the 8 cores (6250 nodes each); every edge is
owned by the core that owns its *start* node, so the per-start segment sums are
core-local and no collective is needed.  Per core, edges are bucketed by
128-node start block and padded to a static layout of 49 blocks x NCH chunks x
128 edges, NCH chosen from the data (ceil(max block edges / 128), even).

Key design points (see earlier versions for the v2 gather design):
  * No dma_gather anywhere: the host materializes [nf[start] ; nf[end]] per
    edge slot as one [128, NCH*128] bf16 stream per block (128B/edge).
    Stage 1 is 2 col-tiled matmuls with the [W1a; W1b] stationary plus one
    for the ef/coord-norm terms.
  * The scatter one-hot is built on-chip with one DVE is_equal per block.
  * Segment-sum: rcg [128 edges, 67] bf16 is the matmul stationary, one-hot
    moving, accumulating aggT [67 feats, 128 nodes] over NCH chunks.
  * Activation-table discipline: only Silu/Tanh/Identity (one table set).
    Gate = 0.5*(1+tanh(0.5 logit + 0.5 b_i)).
  * 1/deg is folded into the coord stream, b_n2 into a DVE add; the nf
    output is written feature-major so no transposes are needed in the
    node-update phase; velocity scales go through a DRAM round-trip to
    avoid 49 PE transposes.
"""
import sys
sys.path.insert(0, "/opt/trn_rl_repo")
import contextlib
import os
import numpy as np

import concourse.bass as bass
import concourse.bacc as bacc
import concourse.mybir as mybir
import concourse.tile as tile
from concourse.bass_utils import run_bass_kernel_spmd

f32 = mybir.dt.float32
bf16 = mybir.dt.bfloat16
AF = mybir.ActivationFunctionType
OP = mybir.AluOpType

# ---- problem constants (hardcoded per contract) ----
N = 50000
E = 1_000_000
H = 64
EF = 16
NCORES = 8
NPC = N // NCORES          # 6250 nodes per core
NBLK = 49                  # 128-node blocks per core (49*128 = 6272 >= 6250)
NPAD = NBLK * 128          # 6272
GRP = 6                    # max chunks per flip/gate group

# CoreSim does not implement Silu; substitute Tanh for structural sim checks.
SIM_ACT = os.environ.get("EGNN_SIMACT", "0") == "1"
AF_SILU = AF.Tanh if SIM_ACT else AF.Silu

_cache = {}


def _sup_widths(half):
    """Supertile widths covering `half` columns (each <= 512, mult of 128)."""
    w = [512] * (half // 512)
    if half % 512:
        w.append(half % 512)
    return w


def _f(x):
    return np.ascontiguousarray(x, np.float32)


def _prep_weights(inp):
    """Small weight/constant tensors, identical on all cores."""
    W_e1 = _f(inp["W_e1"])           # [145, 64]
    w = {}
    w["wse"] = W_e1[0:128].copy()                    # [128, 64] start|end rows
    wef = np.zeros((34, 128), np.float32)
    wef[0:16, 0:64] = W_e1[129:145]
    wef[16:17, 0:64] = W_e1[128:129]
    wef[17:33, 64:128] = W_e1[129:145]
    wef[33:34, 64:128] = W_e1[128:129]
    w["wefcdn"] = wef
    W_e2 = _f(inp["W_e2"]); W_c1 = _f(inp["W_c1"])
    bd = np.zeros((128, 128), np.float32)
    bd[0:64, 0:64] = W_e2; bd[64:128, 64:128] = W_e2
    w["wde2"] = bd
    w["wc1d"] = np.concatenate([W_c1, W_c1], axis=0)   # [128, 64], both halves
    W_i = _f(inp["W_i"]); W_c2 = _f(inp["W_c2"])
    # mcA = [msgA ; chA]  -> out cols: 0:64 msg, 64 gate logit, 65 coord weight
    ra = np.zeros((128, 66), np.float32)
    ra[0:64, 0:64] = np.eye(64); ra[0:64, 64:65] = W_i; ra[64:128, 65:66] = W_c2
    w["rawcA"] = ra
    # mcB = [chB ; msgB]
    rb = np.zeros((128, 66), np.float32)
    rb[64:128, 0:64] = np.eye(64); rb[64:128, 64:65] = W_i; rb[0:64, 65:66] = W_c2
    w["rawcB"] = rb
    w["wn1a"] = _f(inp["W_n1"])[0:64].copy()          # [64, 64] nf rows
    w["wn1b"] = _f(inp["W_n1"])[64:128].copy()        # [64, 64] agg rows
    w["wn2"] = _f(inp["W_n2"])
    w["wv1"] = _f(inp["W_v1"])
    w["wv2"] = _f(inp["W_v2"])
    w["eye3"] = np.eye(3, dtype=np.float32)
    w["arange128"] = np.tile(np.arange(128, dtype=np.float32), (128, 1))

    b_e1 = _f(inp["b_e1"]); b_e2 = _f(inp["b_e2"]); b_c1 = _f(inp["b_c1"])
    w["be1s"] = np.concatenate([b_e1, b_e1]).reshape(128, 1)
    w["be2s"] = np.concatenate([b_e2, b_e2]).reshape(128, 1)
    w["bc1s"] = np.concatenate([b_c1, b_c1]).reshape(128, 1)
    w["bih"] = np.full((128, 1), 0.5 * float(np.asarray(inp["b_i"]).ravel()[0]), np.float32)
    w["bn1c"] = _f(inp["b_n1"]).reshape(64, 1)
    w["bn2c"] = _f(inp["b_n2"]).reshape(64, 1)
    w["bv1c"] = _f(inp["b_v1"]).reshape(64, 1)
    w["bv2c"] = np.full((1, 1), float(np.asarray(inp["b_v2"]).ravel()[0]), np.float32)
    return w


def _prep_core(c, NCH, start, end, ef, nfi, nf_bf, cd_all, cdn_all, invcnt_all):
    bfdt = mybir.dt.np(bf16)
    BLKE = NCH * 128
    ES = NBLK * BLKE
    HALF = BLKE // 2
    lo, hi = c * NPC, (c + 1) * NPC
    sel = (start >= lo) & (start < hi)
    eo = np.nonzero(sel)[0]
    s_loc = (start[eo] - lo).astype(np.int64)
    blk = s_loc >> 7
    order = np.argsort(blk, kind="stable")
    eo = eo[order]; s_loc = s_loc[order]; blk = blk[order]
    counts = np.bincount(blk, minlength=NBLK)
    if counts.max() > BLKE:
        raise RuntimeError(f"block overflow: {counts.max()} > {BLKE}")
    starts_ = np.zeros(NBLK, np.int64)
    starts_[1:] = np.cumsum(counts)[:-1]
    within = np.arange(len(eo)) - starts_[blk]
    slots = blk * BLKE + within

    sg = np.full(ES, lo, np.int64)       # global start per slot (pad -> node lo)
    eg = np.zeros(ES, np.int64)          # global end per slot (pad -> node 0)
    lid = np.full(ES, -1.0, np.float32)
    cds = np.zeros((ES, 3), np.float32)
    cdns = np.zeros(ES, np.float32)
    efs = np.zeros((ES, EF), np.float32)
    sg[slots] = start[eo]
    eg[slots] = end[eo]
    lid[slots] = (s_loc & 127).astype(np.float32)
    cds[slots] = cd_all[eo] * invcnt_all[start[eo]][:, None]
    cdns[slots] = cdn_all[eo]
    efs[slots] = ef[eo]

    d = {}
    # [NBLK, 128, BLKE] bf16: rows 0:64 = nf[start].T, 64:128 = nf[end].T
    nfse = np.empty((ES, 128), bfdt)
    nfse[:, 0:64] = nf_bf[sg]
    nfse[:, 64:128] = nf_bf[eg]
    d["nfse"] = np.ascontiguousarray(
        nfse.reshape(NBLK, BLKE, 128).transpose(0, 2, 1))
    # scatter-orientation lid stream [NBLK, 128, NCH] (edge-in-chunk, chunk)
    lidc = lid.reshape(NBLK, NCH, 128)
    d["lidc"] = np.ascontiguousarray(lidc.transpose(0, 2, 1)).astype(bfdt)
    d["cdem"] = cds.reshape(NBLK, NCH, 128, 3).transpose(0, 2, 1, 3).copy()
    # ef|cdn feature-major: rows 0:17 = A-half edge, 17:34 = B-half edge
    efcdn = np.empty((NBLK, 2, HALF, EF + 1), np.float32)
    efcdn[:, :, :, 0:EF] = efs.reshape(NBLK, 2, HALF, EF)
    efcdn[:, :, :, EF] = cdns.reshape(NBLK, 2, HALF)
    d["efcdn"] = np.ascontiguousarray(
        efcdn.transpose(0, 1, 3, 2).reshape(NBLK, 34, HALF)).astype(bfdt)

    nmc = np.zeros((NPAD, 6), np.float32)
    nmc[0:NPC] = nfi[lo:hi, 0:6]
    d["nodec"] = nmc.reshape(NBLK, 128, 6).transpose(1, 0, 2).reshape(128, NBLK * 6).copy()
    nl = np.zeros((64, NPAD), np.float32)
    nl[:, 0:NPC] = nfi[lo:hi, 6:70].T
    d["nfT_local"] = nl
    return d


def _build_program(NCH):
    STAGE = int(os.environ.get("EGNN_STAGE", "5"))
    BLKE = NCH * 128
    HALF = BLKE // 2
    SUPW = _sup_widths(HALF)        # supertile widths
    SUPO = [sum(SUPW[:i]) for i in range(len(SUPW))]
    # flip/gate groups: chunks [o, o+n) per group
    GRPS = []
    o = 0
    while o < NCH:
        n = min(GRP, NCH - o)
        GRPS.append((o, n))
        o += n

    nc = bacc.Bacc("TRN2", target_bir_lowering=False, debug=False,
                   enable_asserts=False, num_devices=NCORES)

    def din(name, shape, dt=f32):
        return nc.dram_tensor(name, list(shape), dt, kind="ExternalInput").ap()

    nfse_d = din("nfse", [NBLK, 128, BLKE], bf16)
    lidc_d = din("lidc", [NBLK, 128, NCH], bf16)
    cdem_d = din("cdem", [NBLK, 128, NCH, 3])
    efcdn_d = din("efcdn", [NBLK, 34, HALF], bf16)
    nodec_d = din("nodec", [128, NBLK * 6])
    nfT_loc_d = din("nfT_local", [64, NPAD])
    wnames = ["wse", "wefcdn", "wde2", "wc1d", "rawcA", "rawcB",
              "wn1a", "wn1b", "wn2", "wv1", "wv2", "eye3", "arange128",
              "be1s", "be2s", "bc1s", "bih", "bn1c", "bn2c", "bv1c", "bv2c"]
    wshapes = {"wse": [128, 64], "wefcdn": [34, 128],
               "wde2": [128, 128], "wc1d": [128, 64],
               "rawcA": [128, 66], "rawcB": [128, 66],
               "wn1a": [64, 64], "wn1b": [64, 64], "wn2": [64, 64],
               "wv1": [64, 64], "wv2": [64, 1], "eye3": [3, 3],
               "arange128": [128, 128],
               "be1s": [128, 1], "be2s": [128, 1], "bc1s": [128, 1],
               "bih": [128, 1], "bn1c": [64, 1], "bn2c": [64, 1],
               "bv1c": [64, 1], "bv2c": [1, 1]}
    wd = {n: din(n, wshapes[n]) for n in wnames}
    outc_d = nc.dram_tensor("outc", [NPAD, 6], f32, kind="ExternalOutput").ap()
    outT_d = nc.dram_tensor("outT", [64, NPAD], f32, kind="ExternalOutput").ap()
    vs_dram = nc.dram_tensor("vs_dram", [NPAD], f32).ap()

    # weights that are matmul operands in the bf16 pipeline
    BF_W = ("wse", "wefcdn", "wde2", "wc1d", "rawcA", "rawcB", "arange128")

    with tile.TileContext(nc) as tc, contextlib.ExitStack() as ctx:
        wpool = ctx.enter_context(tc.tile_pool(name="w", bufs=1))
        wt = {}
        for n in wnames:
            dt = bf16 if n in BF_W else f32
            t = wpool.tile(wshapes[n], dt, name=f"wt_{n}")
            if dt == f32:
                nc.sync.dma_start(t[:], wd[n][:])
            else:
                tf = wpool.tile(wshapes[n], f32, name=f"wtf_{n}")
                nc.sync.dma_start(tf[:], wd[n][:])
                nc.vector.tensor_copy(t[:], tf[:])
            wt[n] = t
        nodec = wpool.tile([128, NBLK * 6], f32, name="nodec")
        nc.sync.dma_start(nodec[:], nodec_d[:])
        nfT_loc = wpool.tile([64, NPAD], f32, name="nfT_loc")
        nc.sync.dma_start(nfT_loc[:], nfT_loc_d[:])
        vscale = wpool.tile([128, NBLK], f32, name="vscale")
        aggm = wpool.tile([64, NPAD], f32, name="aggm")
        aggc = wpool.tile([3, NPAD], f32, name="aggc")

        # ---------- Phase B: velocity MLP -> vscale [128, NBLK] ----------
        with tc.tile_pool(name="pb", bufs=2) as pb, \
             tc.tile_pool(name="pbp", bufs=2, space="PSUM") as pbp:
            tiles = [(j * 512, 512) for j in range(NPAD // 512)]
            if NPAD % 512:
                tiles.append((NPAD // 512 * 512, NPAD % 512))
            for (o, L) in tiles:
                vps = pbp.tile([64, L], f32, name=f"vps{o}", tag="vps")
                nc.tensor.matmul(vps[:], wt["wv1"][:], nfT_loc[:, o:o + L])
                vh = pb.tile([64, L], f32, name=f"vh{o}", tag="vh")
                nc.scalar.activation(vh[:], vps[:], AF_SILU, bias=wt["bv1c"][:])
                sps = pbp.tile([1, L], f32, name=f"sps{o}", tag="sps")
                nc.tensor.matmul(sps[:], wt["wv2"][:], vh[:])
                vsc = pb.tile([1, L], f32, name=f"vsc{o}", tag="vsc")
                nc.scalar.activation(vsc[:], sps[:], AF.Identity, bias=wt["bv2c"][:])
                nc.sync.dma_start(vs_dram[o:o + L].unsqueeze(0), vsc[:])
            # read back node-block-major: vscale[p, b] = vs_dram[b*128 + p]
            nc.sync.dma_start(vscale[:],
                              vs_dram[:].rearrange("(b p) -> p b", p=128))

        # ---------- Edge sweep (node update fused per block) ----------
        if STAGE >= 2:
            _edge_sweep(nc, tc, STAGE, NCH, SUPW, SUPO, GRPS, wt,
                        nfse_d, lidc_d, cdem_d, efcdn_d, nfT_loc,
                        aggm, aggc, nodec, vscale, outc_d, outT_d)

    nc.compile()
    return nc


def _edge_sweep(nc, tc, STAGE, NCH, SUPW, SUPO, GRPS, wt,
                nfse_d, lidc_d, cdem_d, efcdn_d, nfT_loc,
                aggm, aggc, nodec, vscale, outc_d, outT_d):
    BLKE = NCH * 128
    HALF = BLKE // 2
    NHC = NCH // 2
    NSUP = len(SUPW)

    def mc_of_chunk(mcs, j):
        """Map chunk j to (mc tile, rawc, 128-col slice within supertile)."""
        half, jj = (0, j) if j < NHC else (1, j - NHC)
        col = jj * 128
        for s in range(NSUP):
            if col < SUPO[s] + SUPW[s]:
                off = col - SUPO[s]
                rawc = wt["rawcA"] if half == 0 else wt["rawcB"]
                return mcs[s][half], rawc, slice(off, off + 128)
        raise AssertionError

    with tc.tile_pool(name="pg", bufs=3) as pg, \
         tc.tile_pool(name="ph", bufs=3) as ph, \
         tc.tile_pool(name="pe", bufs=7) as pe, \
         tc.tile_pool(name="pch", bufs=6) as pch, \
         tc.tile_pool(name="poh", bufs=2) as poh, \
         tc.tile_pool(name="px1", bufs=3, space="PSUM") as px1, \
         tc.tile_pool(name="pmc", bufs=2, space="PSUM") as pmc, \
         tc.tile_pool(name="pst", bufs=2, space="PSUM") as pst, \
         tc.tile_pool(name="pagg", bufs=1, space="PSUM") as pagg:

        def phase_c(b):
            cols = slice(b * 128, (b + 1) * 128)
            n1 = pagg.tile([64, 128], f32, name=f"n1{b}", tag="aggT")
            nc.tensor.matmul(n1[:], wt["wn1a"][:], nfT_loc[:, cols],
                             start=True, stop=False)
            nc.tensor.matmul(n1[:], wt["wn1b"][:], aggm[:, cols],
                             start=False, stop=True)
            hn = pch.tile([64, 128], f32, name=f"hn{b}", tag="hn")
            nc.scalar.activation(hn[:], n1[:], AF_SILU, bias=wt["bn1c"][:])
            n2 = pagg.tile([64, 128], f32, name=f"n2{b}", tag="aggT")
            nc.tensor.matmul(n2[:], wt["wn2"][:], hn[:])
            hn2 = pch.tile([64, 128], f32, name=f"hn2{b}", tag="hn2")
            nc.vector.tensor_tensor(
                hn2[:], n2[:], wt["bn2c"][:].broadcast_to([64, 128]), OP.add)
            ot67 = pch.tile([64, 128], f32, name=f"ot67{b}", tag="ot67")
            nc.vector.tensor_tensor(ot67[:], nfT_loc[:, cols], hn2[:], OP.add)
            nc.sync.dma_start(outT_d[:, cols], ot67[:])
            # coords/vels (node-major)
            nmb = nodec[:, b * 6:(b + 1) * 6]
            ctp = pagg.tile([128, 3], f32, name=f"ctp{b}", tag="aggT")
            nc.tensor.transpose(ctp[:], aggc[:, cols], wt["eye3"][:])
            otc = pch.tile([128, 6], f32, name=f"otc{b}", tag="otc")
            t2 = pch.tile([128, 3], f32, name=f"t2{b}", tag="t2")
            nc.vector.tensor_tensor(
                t2[:], nmb[:, 3:6],
                vscale[:, b:b + 1].broadcast_to([128, 3]), OP.mult)
            nc.vector.tensor_copy(otc[:, 3:6], nmb[:, 3:6])
            t3 = pch.tile([128, 3], f32, name=f"t3{b}", tag="t3")
            nc.vector.tensor_tensor(t3[:], ctp[:], t2[:], OP.add)
            nc.vector.tensor_tensor(otc[:, 0:3], t3[:], nmb[:, 0:3], OP.add)
            nc.sync.dma_start(outc_d[b * 128:(b + 1) * 128, :], otc[:])

        state = {}

        def front_half(b):
            nfse = pg.tile([128, BLKE], bf16, name=f"nfse{b}", tag="nfse")
            nc.sync.dma_start(nfse[:], nfse_d[b])
            lidt = pg.tile([128, NCH], bf16, name=f"lidt{b}", tag="lidt")
            nc.sync.dma_start(lidt[:], lidc_d[b])
            cdt = pg.tile([128, NCH, 3], f32, name=f"cdt{b}", tag="cdt")
            nc.sync.dma_start(cdt[:], cdem_d[b])
            eftb = pg.tile([34, HALF], bf16, name=f"eftb{b}", tag="eftb")
            nc.sync.dma_start(eftb[:], efcdn_d[b])

            # scatter one-hot [128 edge-in-chunk, NCH chunk, 128 node] on-chip
            ohts = poh.tile([128, NCH, 128], bf16, name=f"ohts{b}", tag="ohts")
            nc.vector.tensor_tensor(
                ohts[:],
                lidt[:].unsqueeze(2).broadcast_to([128, NCH, 128]),
                wt["arange128"][:].unsqueeze(1).broadcast_to([128, NCH, 128]),
                OP.is_equal)

            if STAGE == 2:
                nc.any.tensor_copy(aggm[:, b * 128:(b + 1) * 128],
                                   ohts[0:64, 0, :])
                return

            # ---- stage 1: x1 accumulation + first silu (per supertile) ----
            h1s = []
            for s in range(NSUP):
                w = SUPW[s]
                sl = slice(SUPO[s], SUPO[s] + w)
                slh = slice(HALF + SUPO[s], HALF + SUPO[s] + w)
                x1 = px1.tile([128, w], f32, name=f"x1{b}_{s}", tag="x1")
                nc.tensor.matmul(x1[0:64, :], wt["wse"][:], nfse[:, sl],
                                 start=True, stop=False, skip_group_check=True)
                nc.tensor.matmul(x1[64:128, :], wt["wse"][:], nfse[:, slh],
                                 start=True, stop=False,
                                 tile_position=(0, 64), skip_group_check=True)
                nc.tensor.matmul(x1[:], wt["wefcdn"][:], eftb[:, sl],
                                 start=False, stop=True, skip_group_check=True)
                h1 = ph.tile([128, w], bf16, name=f"h1{b}_{s}", tag="h1")
                nc.scalar.activation(h1[:], x1[:], AF_SILU, bias=wt["be1s"][:])
                h1s.append(h1)

            # ---- stage 2+3: message silu into mc halves; coord-hidden via
            # two concurrent diagonal-quadrant K=64 matmuls; then ch silu.
            # mcA = [msgA ; chA], mcB = [chB ; msgB]
            mcs = []
            for s in range(NSUP):
                w = SUPW[s]
                mp = pmc.tile([128, w], f32, name=f"mp{b}_{s}", tag="mmid")
                nc.tensor.matmul(mp[:], wt["wde2"][:], h1s[s][:])
                mcA = pe.tile([128, w], bf16, name=f"mcA{b}_{s}", tag="mcA")
                mcB = pe.tile([128, w], bf16, name=f"mcB{b}_{s}", tag="mcB")
                nc.scalar.activation(mcA[0:64, :], mp[0:64, :], AF_SILU,
                                     bias=wt["be2s"][0:64, :])
                nc.scalar.activation(mcB[64:128, :], mp[64:128, :], AF_SILU,
                                     bias=wt["be2s"][64:128, :])
                cpx = pmc.tile([128, w], f32, name=f"cp{b}_{s}", tag="mmid")
                nc.tensor.matmul(cpx[64:128, :], wt["wc1d"][0:64, :], mcA[0:64, :],
                                 start=True, stop=True,
                                 tile_position=(0, 64), skip_group_check=True)
                nc.tensor.matmul(cpx[0:64, :], wt["wc1d"][64:128, :], mcB[64:128, :],
                                 start=True, stop=True,
                                 tile_position=(64, 0), skip_group_check=True)
                nc.scalar.activation(mcA[64:128, :], cpx[64:128, :], AF_SILU,
                                     bias=wt["bc1s"][64:128, :])
                nc.scalar.activation(mcB[0:64, :], cpx[0:64, :], AF_SILU,
                                     bias=wt["bc1s"][0:64, :])
                mcs.append((mcA, mcB))

            state[b] = (mcs, ohts, cdt)
            if STAGE == 3:
                nc.any.tensor_copy(aggm[:, b * 128:(b + 1) * 128],
                                   mcs[0][0][0:64, 0:128])
            return

        def back_half(b):
            mcs, ohts, cdt = state.pop(b)
            # ---- stage 4: edge-major flip (msg|gate|coord) + gate ----
            # rcg [128 edge, NCH chunk, 67]: cols 0:64 msg*gate, 64:67 coord
            rcg = pch.tile([128, NCH, 67], bf16, name=f"rcg{b}", tag="rcg")
            for (go, gn) in GRPS:
                st = pst.tile([128, gn, 66], f32, name=f"st{b}_{go}", tag="st")
                for cg in range(gn):
                    mc, rawc, cc = mc_of_chunk(mcs, go + cg)
                    nc.tensor.matmul(st[:, cg, :], mc[:, cc], rawc[:],
                                     start=True, stop=True)
                jb = slice(go, go + gn)
                tnh = pch.tile([128, gn], f32, name=f"tnh{b}_{go}", tag="tnh")
                nc.scalar.activation(tnh[:], st[:, :, 64:65].squeeze(2),
                                     AF.Tanh, bias=wt["bih"][:], scale=0.5)
                gate = pch.tile([128, gn], f32, name=f"gt{b}_{go}", tag="gate")
                nc.vector.tensor_scalar(out=gate[:], in0=tnh[:], scalar1=1.0,
                                        scalar2=0.5, op0=OP.add, op1=OP.mult)
                nc.vector.tensor_tensor(
                    rcg[:, jb, 0:64], st[:, :, 0:64],
                    gate[:].unsqueeze(2).broadcast_to([128, gn, 64]), OP.mult)
                nc.vector.tensor_tensor(
                    rcg[:, jb, 64:67], cdt[:, jb, :],
                    st[:, :, 65:66].broadcast_to([128, gn, 3]), OP.mult)

            # ---- stage 5: segment-sum scatter into aggT [67, 128 nodes] ----
            # rows 0:64 msg-sum, 64:67 coord-sum
            aggT = pagg.tile([67, 128], f32, name=f"aggT{b}", tag="aggT")
            for j in range(NCH):
                nc.tensor.matmul(aggT[:, :], rcg[:, j, :],
                                 ohts[:, j, :],
                                 start=(j == 0), stop=(j == NCH - 1))
            nc.scalar.activation(aggm[:, b * 128:(b + 1) * 128], aggT[0:64, :],
                                 AF.Identity)
            nc.scalar.activation(aggc[:, b * 128:(b + 1) * 128], aggT[64:67, :],
                                 AF.Identity)

        for b in range(NBLK):
            front_half(b)
            if STAGE >= 4 and b > 0:
                back_half(b - 1)
            if STAGE >= 5 and b > 1:
                phase_c(b - 2)
        if STAGE >= 4:
            back_half(NBLK - 1)
        if STAGE >= 5:
            phase_c(NBLK - 2)
            phase_c(NBLK - 1)


def kernel(**inputs):
    ei = np.asarray(inputs["edge_indices"])
    start = ei[0].astype(np.int64)
    end = ei[1].astype(np.int64)
    ef = _f(inputs["edge_features"])
    nfi = _f(inputs["node_features_input"])
    coords = nfi[:, 0:3]
    cd_all = coords[start] - coords[end]
    cdn_all = np.sqrt((cd_all ** 2).sum(1)).astype(np.float32)
    deg = np.bincount(start, minlength=N).astype(np.float32)
    invcnt_all = (1.0 / np.maximum(deg, 1.0)).astype(np.float32)
    nf_bf = nfi[:, 6:70].astype(mybir.dt.np(bf16))

    # chunk count from data: ceil(max block load / 128), rounded up to even
    mx = 0
    for c in range(NCORES):
        lo, hi = c * NPC, (c + 1) * NPC
        s = start[(start >= lo) & (start < hi)] - lo
        cnt = np.bincount(s >> 7, minlength=NBLK)
        mx = max(mx, int(cnt.max()))
    NCH = -(-mx // 128)
    NCH += NCH % 2
    NCH = max(NCH, 8)

    w = _prep_weights(inputs)

    in_maps = []
    for c in range(NCORES):
        d = _prep_core(c, NCH, start, end, ef, nfi, nf_bf, cd_all, cdn_all,
                       invcnt_all)
        d.update(w)
        in_maps.append(d)

    if _cache.get("NCH") != NCH:
        _cache["NCH"] = NCH
        _cache["nc"] = _build_program(NCH)
    nc = _cache["nc"]
    _cache["in_maps"] = in_maps
    res = run_bass_kernel_spmd(nc, in_maps, list(range(NCORES)))
    out = np.empty((N, 70), np.float32)
    for c in range(NCORES):
        out[c * NPC:(c + 1) * NPC, 0:6] = res.results[c]["outc"][0:NPC]
        out[c * NPC:(c + 1) * NPC, 6:70] = res.results[c]["outT"][:, 0:NPC].T
    return out
